# revision 14
# baseline (speedup 1.0000x reference)
"""Trainium2 Bass kernel for nn_MultiHeadAttention (B=2, S=4096, F=512, H=8, causal mask).

Sharding: 8 cores = 2 (batch) x 4 (head pairs). Each core computes the
projections for its 2 heads, causal flash-style attention with logits in
[Sk, Sq] (transposed) layout, and its partial output projection. The host
pre-transposes q/k/v per batch, slices weights per head pair, and sums the
4 partial outputs per batch afterwards (replaces the all-reduce). Biases are
exact: bq/bk applied on device during PSUM evacuation; bv/bo folded on host
as bv @ wo + bo (valid because softmax rows sum to 1).

The causal structure is not hardcoded blindly: the mask input is classified
on the host into full / partial / skipped [128 x 512] tiles and the device
program is built (and cached) from that schedule, so any 0/1-style additive
mask (including all-zeros) produces a correct program.

Numerics: matmuls run in float32r (tf32-like, full PE rate at free dim
>= 256); walrus requires f32r operands to be produced by rounding
instructions, hence the DVE/GPSIMD convert copies. Softmax uses
exp(logits/8 - 4) with no max pass (logits are O(6) for this problem size;
the -4 offset cancels exactly in the normalization). Denominators ride as a
ones-column in the PV stationary operand and are extracted per S-tile with a
basis-vector matmul (N=2 because f32r requires even free counts).
"""

import numpy as np
from contextlib import ExitStack

import concourse.bass as bass
import concourse.tile as tile
from concourse import bacc, mybir
from concourse import bass2jax

F32 = mybir.dt.float32
F32R = mybir.dt.float32r
BF16 = mybir.dt.bfloat16
AF = mybir.ActivationFunctionType
ALU = mybir.AluOpType

B = 2
S = 4096
NF = 512
NH = 8
D = 64
N_CORES = 8
SQ = 512          # query block width
SK = 128          # key tile height
N_QB = S // SQ    # 8
N_SKT = S // SK   # 32
N_ST = S // 128   # 32 S-tiles for projections / output
SCALE = 1.0 / np.sqrt(np.float32(D))  # 0.125
EXP_BIAS = -4.0   # constant shift inside exp; cancels exactly in normalization

_CACHE: dict = {}


def _classify_mask(mask: np.ndarray):
    """mask: [S, S] additive-style (nonzero => disallowed).

    Returns (schedule, patterns):
      schedule[qb] = list of (sk, qlo, pat_idx_or_None)
      patterns: np.ndarray [n_pat, 128, 512] of multiplicative 0/1 masks.
    """
    m = mask != 0  # True => masked out; indexed [q, k] per the reference
    schedule = []
    patterns = []
    pat_index: dict = {}
    for qb in range(N_QB):
        items = []
        for sk in range(N_SKT):
            # tile in [k, q] orientation to match the on-chip [Sk, Sq] layout
            sub = m[qb * SQ:(qb + 1) * SQ, sk * SK:(sk + 1) * SK].T
            if sub.all():
                continue
            if not sub.any():
                items.append((sk, 0, None))
                continue
            col_full_masked = sub.all(axis=0)
            # first column that is not fully masked
            qlo = int(np.argmax(~col_full_masked))
            # round down to multiple of 128 to keep matmul free dims >= 256-ish
            qlo = (qlo // 128) * 128
            pat = (~sub).astype(np.float32)  # 1 = allowed
            key = pat.tobytes()
            if key not in pat_index:
                pat_index[key] = len(patterns)
                patterns.append(pat)
            items.append((sk, qlo, pat_index[key]))
        schedule.append(tuple(items))
    pats = np.stack(patterns) if patterns else np.ones((1, SK, SQ), np.float32)
    return tuple(schedule), pats


def _group_units(items):
    """Pair up consecutive full tiles for 2-bank exp ops; partial tiles single."""
    units = []
    i = 0
    while i < len(items):
        if (i + 1 < len(items) and items[i][1] == 0 and items[i][2] is None
                and items[i + 1][1] == 0 and items[i + 1][2] is None):
            units.append((items[i], items[i + 1]))
            i += 2
        else:
            units.append((items[i],))
            i += 1
    return units


def _build_program(schedule, n_pat, reps=1):
    nc = bacc.Bacc("TRN2", target_bir_lowering=False, debug=False,
                   num_devices=N_CORES)

    qT = nc.dram_tensor("qT", [NF, S], F32, kind="ExternalInput").ap()
    kT = nc.dram_tensor("kT", [NF, S], F32, kind="ExternalInput").ap()
    vT = nc.dram_tensor("vT", [NF, S], F32, kind="ExternalInput").ap()
    wq_d = nc.dram_tensor("wq", [NF, 128], F32, kind="ExternalInput").ap()
    wk_d = nc.dram_tensor("wk", [NF, 128], F32, kind="ExternalInput").ap()
    wv_d = nc.dram_tensor("wv", [NF, 128], F32, kind="ExternalInput").ap()
    wo_d = nc.dram_tensor("wo", [64, 2, NF], F32, kind="ExternalInput").ap()
    bq_d = nc.dram_tensor("bq", [128, 1], F32, kind="ExternalInput").ap()
    bk_d = nc.dram_tensor("bk", [128, 1], F32, kind="ExternalInput").ap()
    e65_d = nc.dram_tensor("e65", [65, 2], F32, kind="ExternalInput").ap()
    msk_d = nc.dram_tensor("msk", [SK, n_pat * SQ], F32, kind="ExternalInput").ap()
    o_d = nc.dram_tensor("o", [S, NF], F32, kind="ExternalOutput").ap()

    with tile.TileContext(nc) as tc, ExitStack() as octx:
        per = octx.enter_context(tc.tile_pool(name="persist", bufs=1))

        QhT = per.tile([128, S], F32R, tag="qh")      # [head dims (A|B), S]
        KhT = per.tile([128, S], F32R, tag="kh")
        Vaug = per.tile([128, N_SKT, 132], F32R, tag="vaug")  # [A(64)|1|B(64)|1|pad2]
        attnA = per.tile([65, S], F32R, tag="attnA")  # rows 0-63 attn, row 64 denom
        attnB = per.tile([65, S], F32R, tag="attnB")
        wq_st = per.tile([128, 4, 128], F32, tag="wq_st")
        wk_st = per.tile([128, 4, 128], F32, tag="wk_st")
        wq_sb = per.tile([128, 4, 128], F32R, tag="wq")
        wk_sb = per.tile([128, 4, 128], F32R, tag="wk")
        wv_sb = per.tile([128, 4, 128], F32, tag="wv")
        wo_st = per.tile([64, 2, NF], F32, tag="wo_st")
        wo_sb = per.tile([64, 2, NF], F32R, tag="wo")
        bq_sb = per.tile([128, 1], F32, tag="bq")
        bk_sb = per.tile([128, 1], F32, tag="bk")
        e65_st = per.tile([65, 2], F32, tag="e65_st")
        e65_sb = per.tile([65, 2], F32R, tag="e65")
        msk_sb = per.tile([SK, n_pat * SQ], F32, tag="msk")
        ebias = per.tile([128, 1], F32, tag="ebias")
        vones = per.tile([128, N_SKT, 132], F32, tag="vones")

        nc.vector.memset(ebias, EXP_BIAS)
        nc.vector.memset(vones, 1.0)
        nc.vector.tensor_copy(Vaug, vones)
        nc.sync.dma_start(wq_st, wq_d.rearrange("(c p) m -> p c m", p=128))
        nc.sync.dma_start(wk_st, wk_d.rearrange("(c p) m -> p c m", p=128))
        nc.sync.dma_start(wv_sb, wv_d.rearrange("(c p) m -> p c m", p=128))
        nc.vector.tensor_copy(wq_sb, wq_st)
        nc.vector.tensor_copy(wk_sb, wk_st)
        nc.sync.dma_start(bq_sb, bq_d)
        nc.sync.dma_start(bk_sb, bk_d)
        nc.sync.dma_start(msk_sb, msk_d)
        nc.sync.dma_start(wo_st, wo_d)
        nc.vector.tensor_copy(wo_sb, wo_st)
        nc.sync.dma_start(e65_st, e65_d)
        nc.vector.tensor_copy(e65_sb, e65_st)

        for _rep in range(reps):
            # Main pipeline: per query block, project K/Q/V then attention.
            # PSUM banks: qk 1 + v 1 + lt 2x[128,1024]=4 + pv 2 = 8.
            with tc.tile_pool(name="xs", bufs=6) as xs, \
                 tc.tile_pool(name="psqk", bufs=1, space="PSUM") as psqk, \
                 tc.tile_pool(name="psv", bufs=1, space="PSUM") as psvp, \
                 tc.tile_pool(name="pp", bufs=4) as pp, \
                 tc.tile_pool(name="ltp", bufs=2, space="PSUM") as ltp, \
                 tc.tile_pool(name="pvp", bufs=2, space="PSUM") as pvp:
                for qb in range(N_QB):
                    qsl = slice(qb * SQ, (qb + 1) * SQ)
                    q0 = qb * SQ
                    # K/Q projections: f32r via DVE-rounded stream tiles
                    for dst, src, w_s, b_s in ((KhT, kT, wk_sb, bk_sb),
                                               (QhT, qT, wq_sb, bq_sb)):
                        pt = psqk.tile([128, SQ], F32, tag="qk")
                        for f in range(4):
                            xt = xs.tile([128, SQ], F32, tag="x")
                            nc.sync.dma_start(xt, src[f * 128:(f + 1) * 128, qsl])
                            xtr = xs.tile([128, SQ], F32R, tag="xr")
                            nc.gpsimd.tensor_copy(xtr, xt)
                            nc.tensor.matmul(pt, w_s[:, f, :], xtr,
                                             start=(f == 0), stop=(f == 3))
                        nc.vector.tensor_scalar_add(dst[:, qsl], pt, b_s)
                    # V projection for the 4 S-tiles of this block (plain f32)
                    for st in range(4 * qb, 4 * qb + 4):
                        pv_ = psvp.tile([128, 128], F32, tag="v")
                        for f in range(4):
                            xt = xs.tile([128, 128], F32, tag="vx")
                            nc.sync.dma_start(
                                xt, vT[f * 128:(f + 1) * 128,
                                       st * 128:(st + 1) * 128])
                            nc.tensor.matmul(pv_, xt, wv_sb[:, f, :],
                                             start=(f == 0), stop=(f == 3))
                        nc.vector.tensor_copy(
                            Vaug[:, st, 0:130].rearrange(
                                "p (two x) -> p two x", x=65)[:, :, 0:64],
                            pv_.rearrange("p (two x) -> p two x", x=64))
                    # Attention for this block
                    items = schedule[qb]
                    if not items:
                        continue
                    pvA = pvp.tile([65, SQ], F32, tag="pv")
                    pvB = pvp.tile([65, SQ], F32, tag="pv")
                    n_items = len(items)
                    for idx, (sk, qlo, pat) in enumerate(items):
                        ksl = slice(sk * SK, (sk + 1) * SK)
                        qs = slice(q0 + qlo, q0 + SQ)
                        lt = ltp.tile([128, 1024], F32, tag="lt")
                        pAB = pp.tile([128, 1024], F32R, tag="pAB")
                        nc.tensor.matmul(lt[:, qlo:SQ], KhT[0:64, ksl],
                                         QhT[0:64, qs], start=True, stop=True)
                        nc.tensor.matmul(lt[:, SQ + qlo:2 * SQ], KhT[64:128, ksl],
                                         QhT[64:128, qs], start=True, stop=True)
                        if qlo == 0:
                            nc.scalar.activation(pAB, lt, AF.Exp,
                                                 bias=ebias, scale=float(SCALE))
                        else:
                            nc.scalar.activation(pAB[:, qlo:SQ], lt[:, qlo:SQ],
                                                 AF.Exp, bias=ebias,
                                                 scale=float(SCALE))
                            nc.scalar.activation(pAB[:, SQ + qlo:2 * SQ],
                                                 lt[:, SQ + qlo:2 * SQ],
                                                 AF.Exp, bias=ebias,
                                                 scale=float(SCALE))
                        if pat is not None:
                            msl = msk_sb[:, pat * SQ + qlo:(pat + 1) * SQ].bitcast(F32R)
                            nc.vector.tensor_mul(pAB[:, qlo:SQ],
                                                 pAB[:, qlo:SQ], msl)
                            nc.vector.tensor_mul(pAB[:, SQ + qlo:2 * SQ],
                                                 pAB[:, SQ + qlo:2 * SQ], msl)
                        st_flag = (idx == 0)
                        sp_flag = (idx == n_items - 1)
                        nc.tensor.matmul(pvA[:, qlo:SQ], Vaug[:, sk, 0:65],
                                         pAB[:, qlo:SQ],
                                         start=st_flag, stop=sp_flag)
                        nc.tensor.matmul(pvB[:, qlo:SQ], Vaug[:, sk, 65:130],
                                         pAB[:, SQ + qlo:2 * SQ],
                                         start=st_flag, stop=sp_flag)
                    nc.vector.tensor_copy(attnA[:, qsl], pvA)
                    nc.vector.tensor_copy(attnB[:, qsl], pvB)

            # Output projection
            with tc.tile_pool(name="ost", bufs=4) as ost, \
                 tc.tile_pool(name="pso", bufs=4, space="PSUM") as pso, \
                 tc.tile_pool(name="psd", bufs=4, space="PSUM") as psd:
                for st in range(N_ST):
                    sl = slice(st * 128, (st + 1) * 128)
                    oA = pso.tile([128, NF], F32, tag="o")
                    oB = pso.tile([128, NF], F32, tag="o")
                    dA = psd.tile([128, 2], F32, tag="d")
                    dB = psd.tile([128, 2], F32, tag="d")
                    nc.tensor.matmul(oA, attnA[0:64, sl], wo_sb[:, 0, :],
                                     start=True, stop=True)
                    nc.tensor.matmul(oB, attnB[0:64, sl], wo_sb[:, 1, :],
                                     start=True, stop=True)
                    nc.tensor.matmul(dA, attnA[0:65, sl], e65_sb,
                                     start=True, stop=True)
                    nc.tensor.matmul(dB, attnB[0:65, sl], e65_sb,
                                     start=True, stop=True)
                    rA = ost.tile([128, 1], F32, tag="r")
                    rB = ost.tile([128, 1], F32, tag="r")
                    nc.vector.reciprocal(rA, dA[:, 0:1])
                    nc.vector.reciprocal(rB, dB[:, 0:1])
                    t1 = ost.tile([128, NF], F32, tag="t")
                    nc.vector.tensor_scalar_mul(t1, oB, rB)
                    osb = ost.tile([128, NF], F32, tag="os")
                    nc.vector.scalar_tensor_tensor(osb, in0=oA, scalar=rA,
                                                   in1=t1, op0=ALU.mult,
                                                   op1=ALU.add)
                    nc.sync.dma_start(o_d[sl, :], osb)

    nc.compile()
    return nc


def _prep_core_inputs(c, q, k, v, wq, bq, wk, bk, wv, patterns):
    b = c // 4
    hp = c % 4
    cols = slice(128 * hp, 128 * (hp + 1))
    e65 = np.zeros((65, 2), np.float32)
    e65[64, :] = 1.0
    n_pat = patterns.shape[0]
    wo_slice = _prep_core_inputs._wo[cols, :]  # [128, 512]
    return {
        "qT": np.ascontiguousarray(q[b].T),
        "kT": np.ascontiguousarray(k[b].T),
        "vT": np.ascontiguousarray(v[b].T),
        "wq": np.ascontiguousarray(wq[:, cols]),
        "wk": np.ascontiguousarray(wk[:, cols]),
        "wv": np.ascontiguousarray(wv[:, cols]),
        "wo": np.ascontiguousarray(
            wo_slice.reshape(2, 64, NF).transpose(1, 0, 2)),
        "bq": np.ascontiguousarray(bq[cols].reshape(128, 1)),
        "bk": np.ascontiguousarray(bk[cols].reshape(128, 1)),
        "e65": e65,
        "msk": np.ascontiguousarray(
            patterns.transpose(1, 0, 2).reshape(SK, n_pat * SQ)),
    }


def get_state(mask_np, reps=1):
    """Build (or fetch cached) compiled program + schedule for this mask."""
    mask2d = np.asarray(mask_np, dtype=np.float32).reshape(S, S)
    schedule, patterns = _classify_mask(mask2d)
    key = (schedule, patterns.tobytes(), reps)
    if key not in _CACHE:
        nc = _build_program(schedule, patterns.shape[0], reps=reps)
        _CACHE[key] = {"nc": nc, "schedule": schedule, "patterns": patterns}
    return _CACHE[key]


def kernel(q, k, v, mask, wq, bq, wk, bk, wv, bv, wo, bo):
    q = np.asarray(q, np.float32)
    k = np.asarray(k, np.float32)
    v = np.asarray(v, np.float32)
    wq_n = np.asarray(wq, np.float32)
    wk_n = np.asarray(wk, np.float32)
    wv_n = np.asarray(wv, np.float32)
    wo_n = np.asarray(wo, np.float32)
    bq_n = np.asarray(bq, np.float32)
    bk_n = np.asarray(bk, np.float32)
    bv_n = np.asarray(bv, np.float32)
    bo_n = np.asarray(bo, np.float32)

    state = get_state(mask)
    nc = state["nc"]
    patterns = state["patterns"]

    _prep_core_inputs._wo = wo_n
    in_maps = [
        _prep_core_inputs(c, q, k, v, wq_n, bq_n, wk_n, bk_n, wv_n, patterns)
        for c in range(N_CORES)
    ]
    results = bass2jax.run_bass_via_pjrt(nc, in_maps, n_cores=N_CORES)

    bo_eff = bv_n @ wo_n + bo_n  # exact: softmax rows sum to 1
    out = np.empty((B, S, NF), np.float32)
    for b in range(B):
        acc = results[b * 4 + 0]["o"].astype(np.float32)
        for hp in range(1, 4):
            acc = acc + results[b * 4 + hp]["o"]
        out[b] = acc + bo_eff
    return out


# revision 15
# speedup vs baseline: 1.1459x; 1.1459x over previous
"""Trainium2 Bass kernel for nn_MultiHeadAttention (B=2, S=4096, F=512, H=8, causal mask).

Sharding: 8 cores = 2 (batch) x 4 (head pairs). Each core computes the
projections for its 2 heads, causal flash-style attention with logits in
[Sk, Sq] (transposed) layout, and its partial output projection. The host
pre-transposes q/k/v per batch, slices weights per head pair, and sums the
4 partial outputs per batch afterwards (replaces the all-reduce). Biases are
exact: bq/bk applied on device during PSUM evacuation; bv/bo folded on host
as bv @ wo + bo (valid because softmax rows sum to 1).

The causal structure is not hardcoded blindly: the mask input is classified
on the host into full / partial / skipped [128 x 512] tiles and the device
program is built (and cached) from that schedule, so any 0/1-style additive
mask (including all-zeros) produces a correct program.

Numerics: matmuls run in float32r (tf32-like, full PE rate at free dim
>= 256); walrus requires f32r operands to be produced by rounding
instructions, hence the DVE/GPSIMD convert copies. Softmax uses
exp(logits/8 - 4) with no max pass (logits are O(6) for this problem size;
the -4 offset cancels exactly in the normalization). Denominators ride as a
ones-column in the PV stationary operand and are extracted per S-tile with a
basis-vector matmul (N=2 because f32r requires even free counts).
"""

import numpy as np
from contextlib import ExitStack

import concourse.bass as bass
import concourse.tile as tile
from concourse import bacc, mybir
from concourse import bass2jax

F32 = mybir.dt.float32
F32R = mybir.dt.float32r
BF16 = mybir.dt.bfloat16
AF = mybir.ActivationFunctionType
ALU = mybir.AluOpType

B = 2
S = 4096
NF = 512
NH = 8
D = 64
N_CORES = 8
SQ = 512          # query block width
SK = 128          # key tile height
N_QB = S // SQ    # 8
N_SKT = S // SK   # 32
N_ST = S // 128   # 32 S-tiles for projections / output
SCALE = 1.0 / np.sqrt(np.float32(D))  # 0.125
EXP_BIAS = -4.0   # constant shift inside exp; cancels exactly in normalization

_CACHE: dict = {}


def _classify_mask(mask: np.ndarray):
    """mask: [S, S] additive-style (nonzero => disallowed).

    Returns (schedule, patterns):
      schedule[qb] = list of (sk, qlo, pat_idx_or_None)
      patterns: np.ndarray [n_pat, 128, 512] of multiplicative 0/1 masks.
    """
    m = mask != 0  # True => masked out; indexed [q, k] per the reference
    schedule = []
    patterns = []
    pat_index: dict = {}
    for qb in range(N_QB):
        items = []
        for sk in range(N_SKT):
            # tile in [k, q] orientation to match the on-chip [Sk, Sq] layout
            sub = m[qb * SQ:(qb + 1) * SQ, sk * SK:(sk + 1) * SK].T
            if sub.all():
                continue
            if not sub.any():
                items.append((sk, 0, None))
                continue
            col_full_masked = sub.all(axis=0)
            # first column that is not fully masked
            qlo = int(np.argmax(~col_full_masked))
            # round down to multiple of 128 to keep matmul free dims >= 256-ish
            qlo = (qlo // 128) * 128
            pat = (~sub).astype(np.float32)  # 1 = allowed
            key = pat.tobytes()
            if key not in pat_index:
                pat_index[key] = len(patterns)
                patterns.append(pat)
            items.append((sk, qlo, pat_index[key]))
        schedule.append(tuple(items))
    pats = np.stack(patterns) if patterns else np.ones((1, SK, SQ), np.float32)
    return tuple(schedule), pats


def _group_units(items):
    """Pair up consecutive full tiles for 2-bank exp ops; partial tiles single."""
    units = []
    i = 0
    while i < len(items):
        if (i + 1 < len(items) and items[i][1] == 0 and items[i][2] is None
                and items[i + 1][1] == 0 and items[i + 1][2] is None):
            units.append((items[i], items[i + 1]))
            i += 2
        else:
            units.append((items[i],))
            i += 1
    return units


def _build_program(schedule, n_pat, reps=1):
    nc = bacc.Bacc("TRN2", target_bir_lowering=False, debug=False,
                   num_devices=N_CORES)

    qT = nc.dram_tensor("qT", [NF, S], F32, kind="ExternalInput").ap()
    kT = nc.dram_tensor("kT", [NF, S], F32, kind="ExternalInput").ap()
    vT = nc.dram_tensor("vT", [NF, S], F32, kind="ExternalInput").ap()
    wq_d = nc.dram_tensor("wq", [NF, 128], F32, kind="ExternalInput").ap()
    wk_d = nc.dram_tensor("wk", [NF, 128], F32, kind="ExternalInput").ap()
    wv_d = nc.dram_tensor("wv", [NF, 128], F32, kind="ExternalInput").ap()
    wo_d = nc.dram_tensor("wo", [64, 2, NF], F32, kind="ExternalInput").ap()
    bq_d = nc.dram_tensor("bq", [128, 1], F32, kind="ExternalInput").ap()
    bk_d = nc.dram_tensor("bk", [128, 1], F32, kind="ExternalInput").ap()
    e65_d = nc.dram_tensor("e65", [65, 2], F32, kind="ExternalInput").ap()
    msk_d = nc.dram_tensor("msk", [SK, n_pat * SQ], F32, kind="ExternalInput").ap()
    o_d = nc.dram_tensor("o", [S, NF], F32, kind="ExternalOutput").ap()

    with tile.TileContext(nc) as tc, ExitStack() as octx:
        per = octx.enter_context(tc.tile_pool(name="persist", bufs=1))

        QhT = per.tile([128, S], F32R, tag="qh")      # [head dims (A|B), S]
        KhT = per.tile([128, S], F32R, tag="kh")
        Vaug = per.tile([128, N_SKT, 132], F32R, tag="vaug")  # [A(64)|1|B(64)|1|pad2]
        attnA = per.tile([65, S], F32R, tag="attnA")  # rows 0-63 attn, row 64 denom
        attnB = per.tile([65, S], F32R, tag="attnB")
        wq_st = per.tile([128, 4, 128], F32, tag="wq_st")
        wk_st = per.tile([128, 4, 128], F32, tag="wk_st")
        wq_sb = per.tile([128, 4, 128], F32R, tag="wq")
        wk_sb = per.tile([128, 4, 128], F32R, tag="wk")
        wv_sb = per.tile([128, 4, 128], F32, tag="wv")
        wo_st = per.tile([64, 2, NF], F32, tag="wo_st")
        wo_sb = per.tile([64, 2, NF], F32R, tag="wo")
        bq_sb = per.tile([128, 1], F32, tag="bq")
        bk_sb = per.tile([128, 1], F32, tag="bk")
        e65_st = per.tile([65, 2], F32, tag="e65_st")
        e65_sb = per.tile([65, 2], F32R, tag="e65")
        msk_sb = per.tile([SK, n_pat * SQ], F32, tag="msk")
        ebias = per.tile([128, 1], F32, tag="ebias")
        vones = per.tile([128, N_SKT, 132], F32, tag="vones")

        nc.vector.memset(ebias, EXP_BIAS)
        nc.vector.memset(vones, 1.0)
        nc.vector.tensor_copy(Vaug, vones)
        nc.sync.dma_start(wq_st, wq_d.rearrange("(c p) m -> p c m", p=128))
        nc.sync.dma_start(wk_st, wk_d.rearrange("(c p) m -> p c m", p=128))
        nc.sync.dma_start(wv_sb, wv_d.rearrange("(c p) m -> p c m", p=128))
        nc.vector.tensor_copy(wq_sb, wq_st)
        nc.vector.tensor_copy(wk_sb, wk_st)
        nc.sync.dma_start(bq_sb, bq_d)
        nc.sync.dma_start(bk_sb, bk_d)
        nc.sync.dma_start(msk_sb, msk_d)
        nc.sync.dma_start(wo_st, wo_d)
        nc.vector.tensor_copy(wo_sb, wo_st)
        nc.sync.dma_start(e65_st, e65_d)
        nc.vector.tensor_copy(e65_sb, e65_st)

        for _rep in range(reps):
            # Main pipeline: per query block, project K/Q/V then attention.
            # PSUM banks: qk 1 + v 1 + lt 2x[128,1024]=4 + pv 2 = 8.
            with tc.tile_pool(name="xs", bufs=6) as xs, \
                 tc.tile_pool(name="psqk", bufs=1, space="PSUM") as psqk, \
                 tc.tile_pool(name="psv", bufs=1, space="PSUM") as psvp, \
                 tc.tile_pool(name="pp", bufs=4) as pp, \
                 tc.tile_pool(name="ltp", bufs=2, space="PSUM") as ltp, \
                 tc.tile_pool(name="pvp", bufs=2, space="PSUM") as pvp:
                for qb in range(N_QB):
                    qsl = slice(qb * SQ, (qb + 1) * SQ)
                    q0 = qb * SQ
                    # K/Q projections: f32r via DVE-rounded stream tiles
                    for dst, src, w_s, b_s in ((KhT, kT, wk_sb, bk_sb),
                                               (QhT, qT, wq_sb, bq_sb)):
                        pt = psqk.tile([128, SQ], F32, tag="qk")
                        for f in range(4):
                            xt = xs.tile([128, SQ], F32, tag="x")
                            nc.sync.dma_start(xt, src[f * 128:(f + 1) * 128, qsl])
                            xtr = xs.tile([128, SQ], F32R, tag="xr")
                            nc.vector.tensor_copy(xtr, xt)
                            nc.tensor.matmul(pt, w_s[:, f, :], xtr,
                                             start=(f == 0), stop=(f == 3))
                        nc.vector.tensor_scalar_add(dst[:, qsl], pt, b_s)
                    # V projection: one 512-wide load per f-chunk, then the
                    # four 128-wide stationaries slice it (plain f32)
                    vbig = []
                    for f in range(4):
                        xv = xs.tile([128, SQ], F32, tag="vx")
                        nc.sync.dma_start(xv, vT[f * 128:(f + 1) * 128, qsl])
                        vbig.append(xv)
                    for j in range(4):
                        st = 4 * qb + j
                        pv_ = psvp.tile([128, 128], F32, tag="v")
                        for f in range(4):
                            nc.tensor.matmul(pv_, vbig[f][:, j * 128:(j + 1) * 128],
                                             wv_sb[:, f, :],
                                             start=(f == 0), stop=(f == 3))
                        nc.vector.tensor_copy(
                            Vaug[:, st, 0:130].rearrange(
                                "p (two x) -> p two x", x=65)[:, :, 0:64],
                            pv_.rearrange("p (two x) -> p two x", x=64))
                    # Attention for this block
                    items = schedule[qb]
                    if not items:
                        continue
                    pvA = pvp.tile([65, SQ], F32, tag="pv")
                    pvB = pvp.tile([65, SQ], F32, tag="pv")
                    n_items = len(items)
                    for idx, (sk, qlo, pat) in enumerate(items):
                        ksl = slice(sk * SK, (sk + 1) * SK)
                        qs = slice(q0 + qlo, q0 + SQ)
                        lt = ltp.tile([128, 1024], F32, tag="lt")
                        pAB = pp.tile([128, 1024], F32R, tag="pAB")
                        nc.tensor.matmul(lt[:, qlo:SQ], KhT[0:64, ksl],
                                         QhT[0:64, qs], start=True, stop=True)
                        nc.tensor.matmul(lt[:, SQ + qlo:2 * SQ], KhT[64:128, ksl],
                                         QhT[64:128, qs], start=True, stop=True)
                        if qlo == 0:
                            nc.scalar.activation(pAB, lt, AF.Exp,
                                                 bias=ebias, scale=float(SCALE))
                        else:
                            nc.scalar.activation(pAB[:, qlo:SQ], lt[:, qlo:SQ],
                                                 AF.Exp, bias=ebias,
                                                 scale=float(SCALE))
                            nc.scalar.activation(pAB[:, SQ + qlo:2 * SQ],
                                                 lt[:, SQ + qlo:2 * SQ],
                                                 AF.Exp, bias=ebias,
                                                 scale=float(SCALE))
                        if pat is not None:
                            msl = msk_sb[:, pat * SQ + qlo:(pat + 1) * SQ].bitcast(F32R)
                            nc.vector.tensor_mul(pAB[:, qlo:SQ],
                                                 pAB[:, qlo:SQ], msl)
                            nc.vector.tensor_mul(pAB[:, SQ + qlo:2 * SQ],
                                                 pAB[:, SQ + qlo:2 * SQ], msl)
                        st_flag = (idx == 0)
                        sp_flag = (idx == n_items - 1)
                        nc.tensor.matmul(pvA[:, qlo:SQ], Vaug[:, sk, 0:65],
                                         pAB[:, qlo:SQ],
                                         start=st_flag, stop=sp_flag)
                        nc.tensor.matmul(pvB[:, qlo:SQ], Vaug[:, sk, 65:130],
                                         pAB[:, SQ + qlo:2 * SQ],
                                         start=st_flag, stop=sp_flag)
                    nc.vector.tensor_copy(attnA[:, qsl], pvA)
                    nc.vector.tensor_copy(attnB[:, qsl], pvB)

            # Output projection
            with tc.tile_pool(name="ost", bufs=4) as ost, \
                 tc.tile_pool(name="pso", bufs=4, space="PSUM") as pso, \
                 tc.tile_pool(name="psd", bufs=4, space="PSUM") as psd:
                for st in range(N_ST):
                    sl = slice(st * 128, (st + 1) * 128)
                    oA = pso.tile([128, NF], F32, tag="o")
                    oB = pso.tile([128, NF], F32, tag="o")
                    dA = psd.tile([128, 2], F32, tag="d")
                    dB = psd.tile([128, 2], F32, tag="d")
                    nc.tensor.matmul(oA, attnA[0:64, sl], wo_sb[:, 0, :],
                                     start=True, stop=True)
                    nc.tensor.matmul(oB, attnB[0:64, sl], wo_sb[:, 1, :],
                                     start=True, stop=True)
                    nc.tensor.matmul(dA, attnA[0:65, sl], e65_sb,
                                     start=True, stop=True)
                    nc.tensor.matmul(dB, attnB[0:65, sl], e65_sb,
                                     start=True, stop=True)
                    rA = ost.tile([128, 1], F32, tag="r")
                    rB = ost.tile([128, 1], F32, tag="r")
                    nc.vector.reciprocal(rA, dA[:, 0:1])
                    nc.vector.reciprocal(rB, dB[:, 0:1])
                    t1 = ost.tile([128, NF], F32, tag="t")
                    nc.vector.tensor_scalar_mul(t1, oB, rB)
                    osb = ost.tile([128, NF], F32, tag="os")
                    nc.vector.scalar_tensor_tensor(osb, in0=oA, scalar=rA,
                                                   in1=t1, op0=ALU.mult,
                                                   op1=ALU.add)
                    nc.sync.dma_start(o_d[sl, :], osb)

    nc.compile()
    return nc


def _prep_core_inputs(c, q, k, v, wq, bq, wk, bk, wv, patterns):
    b = c // 4
    hp = c % 4
    cols = slice(128 * hp, 128 * (hp + 1))
    e65 = np.zeros((65, 2), np.float32)
    e65[64, :] = 1.0
    n_pat = patterns.shape[0]
    wo_slice = _prep_core_inputs._wo[cols, :]  # [128, 512]
    return {
        "qT": np.ascontiguousarray(q[b].T),
        "kT": np.ascontiguousarray(k[b].T),
        "vT": np.ascontiguousarray(v[b].T),
        "wq": np.ascontiguousarray(wq[:, cols]),
        "wk": np.ascontiguousarray(wk[:, cols]),
        "wv": np.ascontiguousarray(wv[:, cols]),
        "wo": np.ascontiguousarray(
            wo_slice.reshape(2, 64, NF).transpose(1, 0, 2)),
        "bq": np.ascontiguousarray(bq[cols].reshape(128, 1)),
        "bk": np.ascontiguousarray(bk[cols].reshape(128, 1)),
        "e65": e65,
        "msk": np.ascontiguousarray(
            patterns.transpose(1, 0, 2).reshape(SK, n_pat * SQ)),
    }


def get_state(mask_np, reps=1):
    """Build (or fetch cached) compiled program + schedule for this mask."""
    mask2d = np.asarray(mask_np, dtype=np.float32).reshape(S, S)
    schedule, patterns = _classify_mask(mask2d)
    key = (schedule, patterns.tobytes(), reps)
    if key not in _CACHE:
        nc = _build_program(schedule, patterns.shape[0], reps=reps)
        _CACHE[key] = {"nc": nc, "schedule": schedule, "patterns": patterns}
    return _CACHE[key]


def kernel(q, k, v, mask, wq, bq, wk, bk, wv, bv, wo, bo):
    q = np.asarray(q, np.float32)
    k = np.asarray(k, np.float32)
    v = np.asarray(v, np.float32)
    wq_n = np.asarray(wq, np.float32)
    wk_n = np.asarray(wk, np.float32)
    wv_n = np.asarray(wv, np.float32)
    wo_n = np.asarray(wo, np.float32)
    bq_n = np.asarray(bq, np.float32)
    bk_n = np.asarray(bk, np.float32)
    bv_n = np.asarray(bv, np.float32)
    bo_n = np.asarray(bo, np.float32)

    state = get_state(mask)
    nc = state["nc"]
    patterns = state["patterns"]

    _prep_core_inputs._wo = wo_n
    in_maps = [
        _prep_core_inputs(c, q, k, v, wq_n, bq_n, wk_n, bk_n, wv_n, patterns)
        for c in range(N_CORES)
    ]
    results = bass2jax.run_bass_via_pjrt(nc, in_maps, n_cores=N_CORES)

    bo_eff = bv_n @ wo_n + bo_n  # exact: softmax rows sum to 1
    out = np.empty((B, S, NF), np.float32)
    for b in range(B):
        acc = results[b * 4 + 0]["o"].astype(np.float32)
        for hp in range(1, 4):
            acc = acc + results[b * 4 + hp]["o"]
        out[b] = acc + bo_eff
    return out


# revision 16
# speedup vs baseline: 1.1811x; 1.0308x over previous
"""Trainium2 Bass kernel for nn_MultiHeadAttention (B=2, S=4096, F=512, H=8, causal mask).

Sharding: 8 cores = 2 (batch) x 4 (head pairs). Each core computes the
projections for its 2 heads, causal flash-style attention with logits in
[Sk, Sq] (transposed) layout, and its partial output projection. The host
pre-transposes q/k/v per batch, slices weights per head pair, and sums the
4 partial outputs per batch afterwards (replaces the all-reduce). Biases are
exact: bq/bk applied on device during PSUM evacuation; bv/bo folded on host
as bv @ wo + bo (valid because softmax rows sum to 1).

The causal structure is not hardcoded blindly: the mask input is classified
on the host into full / partial / skipped [128 x 512] tiles and the device
program is built (and cached) from that schedule, so any 0/1-style additive
mask (including all-zeros) produces a correct program.

Numerics: matmuls run in float32r (tf32-like, full PE rate at free dim
>= 256); walrus requires f32r operands to be produced by rounding
instructions, hence the DVE/GPSIMD convert copies. Softmax uses
exp(logits/8 - 4) with no max pass (logits are O(6) for this problem size;
the -4 offset cancels exactly in the normalization). Denominators ride as a
ones-column in the PV stationary operand and are extracted per S-tile with a
basis-vector matmul (N=2 because f32r requires even free counts).
"""

import numpy as np
from contextlib import ExitStack

import concourse.bass as bass
import concourse.tile as tile
from concourse import bacc, mybir
from concourse import bass2jax

F32 = mybir.dt.float32
F32R = mybir.dt.float32r
BF16 = mybir.dt.bfloat16
AF = mybir.ActivationFunctionType
ALU = mybir.AluOpType

B = 2
S = 4096
NF = 512
NH = 8
D = 64
N_CORES = 8
SQ = 512          # query block width
SK = 128          # key tile height
N_QB = S // SQ    # 8
N_SKT = S // SK   # 32
N_ST = S // 128   # 32 S-tiles for projections / output
SCALE = 1.0 / np.sqrt(np.float32(D))  # 0.125
EXP_BIAS = -4.0   # constant shift inside exp; cancels exactly in normalization

_CACHE: dict = {}


def _classify_mask(mask: np.ndarray):
    """mask: [S, S] additive-style (nonzero => disallowed).

    Returns (schedule, patterns):
      schedule[qb] = list of (sk, qlo, pat_idx_or_None)
      patterns: np.ndarray [n_pat, 128, 512] of multiplicative 0/1 masks.
    """
    m = mask != 0  # True => masked out; indexed [q, k] per the reference
    schedule = []
    patterns = []
    pat_index: dict = {}
    for qb in range(N_QB):
        items = []
        for sk in range(N_SKT):
            # tile in [k, q] orientation to match the on-chip [Sk, Sq] layout
            sub = m[qb * SQ:(qb + 1) * SQ, sk * SK:(sk + 1) * SK].T
            if sub.all():
                continue
            if not sub.any():
                items.append((sk, 0, None))
                continue
            col_full_masked = sub.all(axis=0)
            # first column that is not fully masked
            qlo = int(np.argmax(~col_full_masked))
            # round down to multiple of 128 to keep matmul free dims >= 256-ish
            qlo = (qlo // 128) * 128
            pat = (~sub).astype(np.float32)  # 1 = allowed
            key = pat.tobytes()
            if key not in pat_index:
                pat_index[key] = len(patterns)
                patterns.append(pat)
            items.append((sk, qlo, pat_index[key]))
        schedule.append(tuple(items))
    pats = np.stack(patterns) if patterns else np.ones((1, SK, SQ), np.float32)
    return tuple(schedule), pats


def _group_units(items):
    """Pair up consecutive full tiles for 2-bank exp ops; partial tiles single."""
    units = []
    i = 0
    while i < len(items):
        if (i + 1 < len(items) and items[i][1] == 0 and items[i][2] is None
                and items[i + 1][1] == 0 and items[i + 1][2] is None):
            units.append((items[i], items[i + 1]))
            i += 2
        else:
            units.append((items[i],))
            i += 1
    return units


def _build_program(schedule, n_pat, reps=1):
    nc = bacc.Bacc("TRN2", target_bir_lowering=False, debug=False,
                   num_devices=N_CORES)

    qT = nc.dram_tensor("qT", [NF, S], F32, kind="ExternalInput").ap()
    kT = nc.dram_tensor("kT", [NF, S], F32, kind="ExternalInput").ap()
    vT = nc.dram_tensor("vT", [NF, S], F32, kind="ExternalInput").ap()
    wq_d = nc.dram_tensor("wq", [NF, 128], F32, kind="ExternalInput").ap()
    wk_d = nc.dram_tensor("wk", [NF, 128], F32, kind="ExternalInput").ap()
    wv_d = nc.dram_tensor("wv", [NF, 128], F32, kind="ExternalInput").ap()
    wo_d = nc.dram_tensor("wo", [64, 2, NF], F32, kind="ExternalInput").ap()
    bq_d = nc.dram_tensor("bq", [128, 1], F32, kind="ExternalInput").ap()
    bk_d = nc.dram_tensor("bk", [128, 1], F32, kind="ExternalInput").ap()
    e65_d = nc.dram_tensor("e65", [65, 2], F32, kind="ExternalInput").ap()
    msk_d = nc.dram_tensor("msk", [SK, n_pat * SQ], F32, kind="ExternalInput").ap()
    o_d = nc.dram_tensor("o", [S, NF], F32, kind="ExternalOutput").ap()

    with tile.TileContext(nc) as tc, ExitStack() as octx:
        per = octx.enter_context(tc.tile_pool(name="persist", bufs=1))

        QhT = per.tile([128, S], F32R, tag="qh")      # [head dims (A|B), S]
        KhT = per.tile([128, S], F32R, tag="kh")
        Vaug = per.tile([128, N_SKT, 132], F32R, tag="vaug")  # [A(64)|1|B(64)|1|pad2]
        attnA = per.tile([65, S], F32R, tag="attnA")  # rows 0-63 attn, row 64 denom
        attnB = per.tile([65, S], F32R, tag="attnB")
        wq_st = per.tile([128, 4, 128], F32, tag="wq_st")
        wk_st = per.tile([128, 4, 128], F32, tag="wk_st")
        wq_sb = per.tile([128, 4, 128], F32R, tag="wq")
        wk_sb = per.tile([128, 4, 128], F32R, tag="wk")
        wv_sb = per.tile([128, 4, 128], F32, tag="wv")
        wo_st = per.tile([64, 2, NF], F32, tag="wo_st")
        wo_sb = per.tile([64, 2, NF], F32R, tag="wo")
        bq_sb = per.tile([128, 1], F32, tag="bq")
        bk_sb = per.tile([128, 1], F32, tag="bk")
        e65_st = per.tile([65, 2], F32, tag="e65_st")
        e65_sb = per.tile([65, 2], F32R, tag="e65")
        msk_sb = per.tile([SK, n_pat * SQ], F32, tag="msk")
        ebias = per.tile([128, 1], F32, tag="ebias")
        vones = per.tile([128, N_SKT, 132], F32, tag="vones")

        nc.vector.memset(ebias, EXP_BIAS)
        nc.vector.memset(vones, 1.0)
        nc.vector.tensor_copy(Vaug, vones)
        nc.sync.dma_start(wq_st, wq_d.rearrange("(c p) m -> p c m", p=128))
        nc.sync.dma_start(wk_st, wk_d.rearrange("(c p) m -> p c m", p=128))
        nc.sync.dma_start(wv_sb, wv_d.rearrange("(c p) m -> p c m", p=128))
        nc.vector.tensor_copy(wq_sb, wq_st)
        nc.vector.tensor_copy(wk_sb, wk_st)
        nc.sync.dma_start(bq_sb, bq_d)
        nc.sync.dma_start(bk_sb, bk_d)
        nc.sync.dma_start(msk_sb, msk_d)
        nc.sync.dma_start(wo_st, wo_d)
        nc.vector.tensor_copy(wo_sb, wo_st)
        nc.sync.dma_start(e65_st, e65_d)
        nc.vector.tensor_copy(e65_sb, e65_st)

        for _rep in range(reps):
            # Main pipeline: per query block, project K/Q/V then attention.
            # PSUM banks: qk 1 + v 1 + lt 2x[128,1024]=4 + pv 2 = 8.
            with tc.tile_pool(name="xs", bufs=2) as xs, \
                 tc.tile_pool(name="psqk", bufs=1, space="PSUM") as psqk, \
                 tc.tile_pool(name="psv", bufs=1, space="PSUM") as psvp, \
                 tc.tile_pool(name="pp", bufs=4) as pp, \
                 tc.tile_pool(name="ltp", bufs=2, space="PSUM") as ltp, \
                 tc.tile_pool(name="pvp", bufs=2, space="PSUM") as pvp:
                for qb in range(N_QB):
                    qsl = slice(qb * SQ, (qb + 1) * SQ)
                    q0 = qb * SQ
                    # K/Q projections: one merged 1MB load + one rounding
                    # copy per tensor per block
                    for dst, src, w_s, b_s in ((KhT, kT, wk_sb, bk_sb),
                                               (QhT, qT, wq_sb, bq_sb)):
                        pt = psqk.tile([128, SQ], F32, tag="qk")
                        xb = xs.tile([128, 4, SQ], F32, tag="x")
                        nc.sync.dma_start(
                            xb, src.rearrange("(c p) m -> p c m", p=128)[:, :, qsl])
                        xr = xs.tile([128, 4, SQ], F32R, tag="xr")
                        nc.vector.tensor_copy(xr, xb)
                        for f in range(4):
                            nc.tensor.matmul(pt, w_s[:, f, :], xr[:, f, :],
                                             start=(f == 0), stop=(f == 3))
                        nc.vector.tensor_scalar_add(dst[:, qsl], pt, b_s)
                    # V projection: one 512-wide load per f-chunk, then the
                    # four 128-wide stationaries slice it (plain f32)
                    vbig = xs.tile([128, 4, SQ], F32, tag="vx")
                    nc.sync.dma_start(
                        vbig, vT.rearrange("(c p) m -> p c m", p=128)[:, :, qsl])
                    for j in range(4):
                        st = 4 * qb + j
                        pv_ = psvp.tile([128, 128], F32, tag="v")
                        for f in range(4):
                            nc.tensor.matmul(pv_, vbig[:, f, j * 128:(j + 1) * 128],
                                             wv_sb[:, f, :],
                                             start=(f == 0), stop=(f == 3))
                        nc.vector.tensor_copy(
                            Vaug[:, st, 0:130].rearrange(
                                "p (two x) -> p two x", x=65)[:, :, 0:64],
                            pv_.rearrange("p (two x) -> p two x", x=64))
                    # Attention for this block
                    items = schedule[qb]
                    if not items:
                        continue
                    pvA = pvp.tile([65, SQ], F32, tag="pv")
                    pvB = pvp.tile([65, SQ], F32, tag="pv")
                    n_items = len(items)
                    for idx, (sk, qlo, pat) in enumerate(items):
                        ksl = slice(sk * SK, (sk + 1) * SK)
                        qs = slice(q0 + qlo, q0 + SQ)
                        lt = ltp.tile([128, 1024], F32, tag="lt")
                        pAB = pp.tile([128, 1024], F32R, tag="pAB")
                        nc.tensor.matmul(lt[:, qlo:SQ], KhT[0:64, ksl],
                                         QhT[0:64, qs], start=True, stop=True)
                        nc.tensor.matmul(lt[:, SQ + qlo:2 * SQ], KhT[64:128, ksl],
                                         QhT[64:128, qs], start=True, stop=True)
                        if qlo == 0:
                            nc.scalar.activation(pAB, lt, AF.Exp,
                                                 bias=ebias, scale=float(SCALE))
                        else:
                            nc.scalar.activation(pAB[:, qlo:SQ], lt[:, qlo:SQ],
                                                 AF.Exp, bias=ebias,
                                                 scale=float(SCALE))
                            nc.scalar.activation(pAB[:, SQ + qlo:2 * SQ],
                                                 lt[:, SQ + qlo:2 * SQ],
                                                 AF.Exp, bias=ebias,
                                                 scale=float(SCALE))
                        if pat is not None:
                            msl = msk_sb[:, pat * SQ + qlo:(pat + 1) * SQ].bitcast(F32R)
                            nc.vector.tensor_mul(pAB[:, qlo:SQ],
                                                 pAB[:, qlo:SQ], msl)
                            nc.vector.tensor_mul(pAB[:, SQ + qlo:2 * SQ],
                                                 pAB[:, SQ + qlo:2 * SQ], msl)
                        st_flag = (idx == 0)
                        sp_flag = (idx == n_items - 1)
                        nc.tensor.matmul(pvA[:, qlo:SQ], Vaug[:, sk, 0:65],
                                         pAB[:, qlo:SQ],
                                         start=st_flag, stop=sp_flag)
                        nc.tensor.matmul(pvB[:, qlo:SQ], Vaug[:, sk, 65:130],
                                         pAB[:, SQ + qlo:2 * SQ],
                                         start=st_flag, stop=sp_flag)
                    nc.vector.tensor_copy(attnA[:, qsl], pvA)
                    nc.vector.tensor_copy(attnB[:, qsl], pvB)

            # Output projection
            with tc.tile_pool(name="ost", bufs=4) as ost, \
                 tc.tile_pool(name="pso", bufs=4, space="PSUM") as pso, \
                 tc.tile_pool(name="psd", bufs=4, space="PSUM") as psd:
                for st in range(N_ST):
                    sl = slice(st * 128, (st + 1) * 128)
                    oA = pso.tile([128, NF], F32, tag="o")
                    oB = pso.tile([128, NF], F32, tag="o")
                    dA = psd.tile([128, 2], F32, tag="d")
                    dB = psd.tile([128, 2], F32, tag="d")
                    nc.tensor.matmul(oA, attnA[0:64, sl], wo_sb[:, 0, :],
                                     start=True, stop=True)
                    nc.tensor.matmul(oB, attnB[0:64, sl], wo_sb[:, 1, :],
                                     start=True, stop=True)
                    nc.tensor.matmul(dA, attnA[0:65, sl], e65_sb,
                                     start=True, stop=True)
                    nc.tensor.matmul(dB, attnB[0:65, sl], e65_sb,
                                     start=True, stop=True)
                    rA = ost.tile([128, 1], F32, tag="r")
                    rB = ost.tile([128, 1], F32, tag="r")
                    nc.vector.reciprocal(rA, dA[:, 0:1])
                    nc.vector.reciprocal(rB, dB[:, 0:1])
                    t1 = ost.tile([128, NF], F32, tag="t")
                    nc.vector.tensor_scalar_mul(t1, oB, rB)
                    osb = ost.tile([128, NF], F32, tag="os")
                    nc.vector.scalar_tensor_tensor(osb, in0=oA, scalar=rA,
                                                   in1=t1, op0=ALU.mult,
                                                   op1=ALU.add)
                    nc.sync.dma_start(o_d[sl, :], osb)

    nc.compile()
    return nc


def _prep_core_inputs(c, q, k, v, wq, bq, wk, bk, wv, patterns):
    b = c // 4
    hp = c % 4
    cols = slice(128 * hp, 128 * (hp + 1))
    e65 = np.zeros((65, 2), np.float32)
    e65[64, :] = 1.0
    n_pat = patterns.shape[0]
    wo_slice = _prep_core_inputs._wo[cols, :]  # [128, 512]
    return {
        "qT": np.ascontiguousarray(q[b].T),
        "kT": np.ascontiguousarray(k[b].T),
        "vT": np.ascontiguousarray(v[b].T),
        "wq": np.ascontiguousarray(wq[:, cols]),
        "wk": np.ascontiguousarray(wk[:, cols]),
        "wv": np.ascontiguousarray(wv[:, cols]),
        "wo": np.ascontiguousarray(
            wo_slice.reshape(2, 64, NF).transpose(1, 0, 2)),
        "bq": np.ascontiguousarray(bq[cols].reshape(128, 1)),
        "bk": np.ascontiguousarray(bk[cols].reshape(128, 1)),
        "e65": e65,
        "msk": np.ascontiguousarray(
            patterns.transpose(1, 0, 2).reshape(SK, n_pat * SQ)),
    }


def get_state(mask_np, reps=1):
    """Build (or fetch cached) compiled program + schedule for this mask."""
    mask2d = np.asarray(mask_np, dtype=np.float32).reshape(S, S)
    schedule, patterns = _classify_mask(mask2d)
    key = (schedule, patterns.tobytes(), reps)
    if key not in _CACHE:
        nc = _build_program(schedule, patterns.shape[0], reps=reps)
        _CACHE[key] = {"nc": nc, "schedule": schedule, "patterns": patterns}
    return _CACHE[key]


def kernel(q, k, v, mask, wq, bq, wk, bk, wv, bv, wo, bo):
    q = np.asarray(q, np.float32)
    k = np.asarray(k, np.float32)
    v = np.asarray(v, np.float32)
    wq_n = np.asarray(wq, np.float32)
    wk_n = np.asarray(wk, np.float32)
    wv_n = np.asarray(wv, np.float32)
    wo_n = np.asarray(wo, np.float32)
    bq_n = np.asarray(bq, np.float32)
    bk_n = np.asarray(bk, np.float32)
    bv_n = np.asarray(bv, np.float32)
    bo_n = np.asarray(bo, np.float32)

    state = get_state(mask)
    nc = state["nc"]
    patterns = state["patterns"]

    _prep_core_inputs._wo = wo_n
    in_maps = [
        _prep_core_inputs(c, q, k, v, wq_n, bq_n, wk_n, bk_n, wv_n, patterns)
        for c in range(N_CORES)
    ]
    results = bass2jax.run_bass_via_pjrt(nc, in_maps, n_cores=N_CORES)

    bo_eff = bv_n @ wo_n + bo_n  # exact: softmax rows sum to 1
    out = np.empty((B, S, NF), np.float32)
    for b in range(B):
        acc = results[b * 4 + 0]["o"].astype(np.float32)
        for hp in range(1, 4):
            acc = acc + results[b * 4 + hp]["o"]
        out[b] = acc + bo_eff
    return out


# revision 17
# speedup vs baseline: 1.1860x; 1.0041x over previous
"""Trainium2 Bass kernel for nn_MultiHeadAttention (B=2, S=4096, F=512, H=8, causal mask).

Sharding: 8 cores = 2 (batch) x 4 (head pairs). Each core computes the
projections for its 2 heads, causal flash-style attention with logits in
[Sk, Sq] (transposed) layout, and its partial output projection. The host
pre-transposes q/k/v per batch, slices weights per head pair, and sums the
4 partial outputs per batch afterwards (replaces the all-reduce). Biases are
exact: bq/bk applied on device during PSUM evacuation; bv/bo folded on host
as bv @ wo + bo (valid because softmax rows sum to 1).

The causal structure is not hardcoded blindly: the mask input is classified
on the host into full / partial / skipped [128 x 512] tiles and the device
program is built (and cached) from that schedule, so any 0/1-style additive
mask (including all-zeros) produces a correct program.

Numerics: matmuls run in float32r (tf32-like, full PE rate at free dim
>= 256); walrus requires f32r operands to be produced by rounding
instructions, hence the DVE/GPSIMD convert copies. Softmax uses
exp(logits/8 - 4) with no max pass (logits are O(6) for this problem size;
the -4 offset cancels exactly in the normalization). Denominators ride as a
ones-column in the PV stationary operand and are extracted per S-tile with a
basis-vector matmul (N=2 because f32r requires even free counts).
"""

import numpy as np
from contextlib import ExitStack

import concourse.bass as bass
import concourse.tile as tile
from concourse import bacc, mybir
from concourse import bass2jax

F32 = mybir.dt.float32
F32R = mybir.dt.float32r
BF16 = mybir.dt.bfloat16
AF = mybir.ActivationFunctionType
ALU = mybir.AluOpType

B = 2
S = 4096
NF = 512
NH = 8
D = 64
N_CORES = 8
SQ = 512          # query block width
SK = 128          # key tile height
N_QB = S // SQ    # 8
N_SKT = S // SK   # 32
N_ST = S // 128   # 32 S-tiles for projections / output
SCALE = 1.0 / np.sqrt(np.float32(D))  # 0.125
EXP_BIAS = -4.0   # constant shift inside exp; cancels exactly in normalization

_CACHE: dict = {}


def _classify_mask(mask: np.ndarray):
    """mask: [S, S] additive-style (nonzero => disallowed).

    Returns (schedule, patterns):
      schedule[qb] = list of (sk, qlo, pat_idx_or_None)
      patterns: np.ndarray [n_pat, 128, 512] of multiplicative 0/1 masks.
    """
    m = mask != 0  # True => masked out; indexed [q, k] per the reference
    schedule = []
    patterns = []
    pat_index: dict = {}
    for qb in range(N_QB):
        items = []
        for sk in range(N_SKT):
            # tile in [k, q] orientation to match the on-chip [Sk, Sq] layout
            sub = m[qb * SQ:(qb + 1) * SQ, sk * SK:(sk + 1) * SK].T
            if sub.all():
                continue
            if not sub.any():
                items.append((sk, 0, None))
                continue
            col_full_masked = sub.all(axis=0)
            # first column that is not fully masked
            qlo = int(np.argmax(~col_full_masked))
            # round down to multiple of 128 to keep matmul free dims >= 256-ish
            qlo = (qlo // 128) * 128
            pat = (~sub).astype(np.float32)  # 1 = allowed
            key = pat.tobytes()
            if key not in pat_index:
                pat_index[key] = len(patterns)
                patterns.append(pat)
            items.append((sk, qlo, pat_index[key]))
        schedule.append(tuple(items))
    pats = np.stack(patterns) if patterns else np.ones((1, SK, SQ), np.float32)
    return tuple(schedule), pats


def _group_units(items):
    """Pair up consecutive full tiles for 2-bank exp ops; partial tiles single."""
    units = []
    i = 0
    while i < len(items):
        if (i + 1 < len(items) and items[i][1] == 0 and items[i][2] is None
                and items[i + 1][1] == 0 and items[i + 1][2] is None):
            units.append((items[i], items[i + 1]))
            i += 2
        else:
            units.append((items[i],))
            i += 1
    return units


def _build_program(schedule, n_pat, reps=1):
    nc = bacc.Bacc("TRN2", target_bir_lowering=False, debug=False,
                   num_devices=N_CORES)

    qT = nc.dram_tensor("qT", [NF, S], F32, kind="ExternalInput").ap()
    kT = nc.dram_tensor("kT", [NF, S], F32, kind="ExternalInput").ap()
    vT = nc.dram_tensor("vT", [NF, S], F32, kind="ExternalInput").ap()
    wq_d = nc.dram_tensor("wq", [NF, 128], F32, kind="ExternalInput").ap()
    wk_d = nc.dram_tensor("wk", [NF, 128], F32, kind="ExternalInput").ap()
    wv_d = nc.dram_tensor("wv", [NF, 128], F32, kind="ExternalInput").ap()
    wo_d = nc.dram_tensor("wo", [64, 2, NF], F32, kind="ExternalInput").ap()
    bq_d = nc.dram_tensor("bq", [128, 1], F32, kind="ExternalInput").ap()
    bk_d = nc.dram_tensor("bk", [128, 1], F32, kind="ExternalInput").ap()
    e65_d = nc.dram_tensor("e65", [65, 2], F32, kind="ExternalInput").ap()
    msk_d = nc.dram_tensor("msk", [SK, n_pat * SQ], F32, kind="ExternalInput").ap()
    o_d = nc.dram_tensor("o", [S, NF], F32, kind="ExternalOutput").ap()

    with tile.TileContext(nc) as tc, ExitStack() as octx:
        per = octx.enter_context(tc.tile_pool(name="persist", bufs=1))

        QhT = per.tile([128, S], F32R, tag="qh")      # [head dims (A|B), S]
        KhT = per.tile([128, S], F32R, tag="kh")
        Vaug = per.tile([128, N_SKT, 132], F32R, tag="vaug")  # [A(64)|1|B(64)|1|pad2]
        attnA = per.tile([65, S], F32R, tag="attnA")  # rows 0-63 attn, row 64 denom
        attnB = per.tile([65, S], F32R, tag="attnB")
        wq_st = per.tile([128, 4, 128], F32, tag="wq_st")
        wk_st = per.tile([128, 4, 128], F32, tag="wk_st")
        wq_sb = per.tile([128, 4, 128], F32R, tag="wq")
        wk_sb = per.tile([128, 4, 128], F32R, tag="wk")
        wv_sb = per.tile([128, 4, 128], F32, tag="wv")
        wo_st = per.tile([64, 2, NF], F32, tag="wo_st")
        wo_sb = per.tile([64, 2, NF], F32R, tag="wo")
        bq_sb = per.tile([128, 1], F32, tag="bq")
        bk_sb = per.tile([128, 1], F32, tag="bk")
        e65_st = per.tile([65, 2], F32, tag="e65_st")
        e65_sb = per.tile([65, 2], F32R, tag="e65")
        msk_sb = per.tile([SK, n_pat * SQ], F32, tag="msk")
        ebias = per.tile([128, 1], F32, tag="ebias")
        vones = per.tile([128, N_SKT, 132], F32, tag="vones")

        nc.vector.memset(ebias, EXP_BIAS)
        nc.vector.memset(vones, 1.0)
        nc.vector.tensor_copy(Vaug, vones)
        nc.sync.dma_start(wq_st, wq_d.rearrange("(c p) m -> p c m", p=128))
        nc.sync.dma_start(wk_st, wk_d.rearrange("(c p) m -> p c m", p=128))
        nc.sync.dma_start(wv_sb, wv_d.rearrange("(c p) m -> p c m", p=128))
        nc.vector.tensor_copy(wq_sb, wq_st)
        nc.vector.tensor_copy(wk_sb, wk_st)
        nc.sync.dma_start(bq_sb, bq_d)
        nc.sync.dma_start(bk_sb, bk_d)
        nc.sync.dma_start(msk_sb, msk_d)
        nc.sync.dma_start(wo_st, wo_d)
        nc.vector.tensor_copy(wo_sb, wo_st)
        nc.sync.dma_start(e65_st, e65_d)
        nc.vector.tensor_copy(e65_sb, e65_st)

        for _rep in range(reps):
            # Main pipeline: per query block, project K/Q/V then attention.
            # PSUM banks: qk 1 + v 1 + lt 2x[128,1024]=4 + pv 2 = 8.
            with tc.tile_pool(name="xs", bufs=2) as xs, \
                 tc.tile_pool(name="psqk", bufs=1, space="PSUM") as psqk, \
                 tc.tile_pool(name="psv", bufs=1, space="PSUM") as psvp, \
                 tc.tile_pool(name="pp", bufs=4) as pp, \
                 tc.tile_pool(name="ltp", bufs=2, space="PSUM") as ltp, \
                 tc.tile_pool(name="pvp", bufs=2, space="PSUM") as pvp:
                for qb in range(N_QB):
                    qsl = slice(qb * SQ, (qb + 1) * SQ)
                    q0 = qb * SQ
                    # K/Q projections: one merged 1MB load + one rounding
                    # copy per tensor per block
                    for dst, src, w_s, b_s in ((KhT, kT, wk_sb, bk_sb),
                                               (QhT, qT, wq_sb, bq_sb)):
                        pt = psqk.tile([128, SQ], F32, tag="qk")
                        xb = xs.tile([128, 4, SQ], F32, tag="x")
                        nc.sync.dma_start(
                            xb, src.rearrange("(c p) m -> p c m", p=128)[:, :, qsl])
                        xr = xs.tile([128, 4, SQ], F32R, tag="xr")
                        nc.vector.tensor_copy(xr, xb)
                        for f in range(4):
                            nc.tensor.matmul(pt, w_s[:, f, :], xr[:, f, :],
                                             start=(f == 0), stop=(f == 3))
                        nc.vector.tensor_scalar_add(dst[:, qsl], pt, b_s)
                    # V projection: one 512-wide load per f-chunk, then the
                    # four 128-wide stationaries slice it (plain f32)
                    vbig = xs.tile([128, 4, SQ], F32, tag="vx")
                    nc.gpsimd.dma_start(
                        vbig, vT.rearrange("(c p) m -> p c m", p=128)[:, :, qsl])
                    for j in range(4):
                        st = 4 * qb + j
                        pv_ = psvp.tile([128, 128], F32, tag="v")
                        for f in range(4):
                            nc.tensor.matmul(pv_, vbig[:, f, j * 128:(j + 1) * 128],
                                             wv_sb[:, f, :],
                                             start=(f == 0), stop=(f == 3))
                        nc.vector.tensor_copy(
                            Vaug[:, st, 0:130].rearrange(
                                "p (two x) -> p two x", x=65)[:, :, 0:64],
                            pv_.rearrange("p (two x) -> p two x", x=64))
                    # Attention for this block
                    items = schedule[qb]
                    if not items:
                        continue
                    pvA = pvp.tile([65, SQ], F32, tag="pv")
                    pvB = pvp.tile([65, SQ], F32, tag="pv")
                    n_items = len(items)
                    for idx, (sk, qlo, pat) in enumerate(items):
                        ksl = slice(sk * SK, (sk + 1) * SK)
                        qs = slice(q0 + qlo, q0 + SQ)
                        lt = ltp.tile([128, 1024], F32, tag="lt")
                        pAB = pp.tile([128, 1024], F32R, tag="pAB")
                        nc.tensor.matmul(lt[:, qlo:SQ], KhT[0:64, ksl],
                                         QhT[0:64, qs], start=True, stop=True)
                        nc.tensor.matmul(lt[:, SQ + qlo:2 * SQ], KhT[64:128, ksl],
                                         QhT[64:128, qs], start=True, stop=True)
                        if qlo == 0:
                            nc.scalar.activation(pAB, lt, AF.Exp,
                                                 bias=ebias, scale=float(SCALE))
                        else:
                            oap = pAB.rearrange("p (two q) -> p two q",
                                                q=SQ)[:, :, qlo:SQ]
                            iap = lt.rearrange("p (two q) -> p two q",
                                               q=SQ)[:, :, qlo:SQ]
                            nc.scalar.activation(oap, iap, AF.Exp,
                                                 bias=ebias, scale=float(SCALE))
                        if pat is not None:
                            msl = msk_sb[:, pat * SQ + qlo:(pat + 1) * SQ].bitcast(F32R)
                            nc.vector.tensor_mul(pAB[:, qlo:SQ],
                                                 pAB[:, qlo:SQ], msl)
                            nc.vector.tensor_mul(pAB[:, SQ + qlo:2 * SQ],
                                                 pAB[:, SQ + qlo:2 * SQ], msl)
                        st_flag = (idx == 0)
                        sp_flag = (idx == n_items - 1)
                        nc.tensor.matmul(pvA[:, qlo:SQ], Vaug[:, sk, 0:65],
                                         pAB[:, qlo:SQ],
                                         start=st_flag, stop=sp_flag)
                        nc.tensor.matmul(pvB[:, qlo:SQ], Vaug[:, sk, 65:130],
                                         pAB[:, SQ + qlo:2 * SQ],
                                         start=st_flag, stop=sp_flag)
                    nc.vector.tensor_copy(attnA[:, qsl], pvA)
                    nc.vector.tensor_copy(attnB[:, qsl], pvB)

            # Output projection
            with tc.tile_pool(name="ost", bufs=4) as ost, \
                 tc.tile_pool(name="pso", bufs=4, space="PSUM") as pso, \
                 tc.tile_pool(name="psd", bufs=4, space="PSUM") as psd:
                for st in range(N_ST):
                    sl = slice(st * 128, (st + 1) * 128)
                    oA = pso.tile([128, NF], F32, tag="o")
                    oB = pso.tile([128, NF], F32, tag="o")
                    dA = psd.tile([128, 2], F32, tag="d")
                    dB = psd.tile([128, 2], F32, tag="d")
                    nc.tensor.matmul(oA, attnA[0:64, sl], wo_sb[:, 0, :],
                                     start=True, stop=True)
                    nc.tensor.matmul(oB, attnB[0:64, sl], wo_sb[:, 1, :],
                                     start=True, stop=True)
                    nc.tensor.matmul(dA, attnA[0:65, sl], e65_sb,
                                     start=True, stop=True)
                    nc.tensor.matmul(dB, attnB[0:65, sl], e65_sb,
                                     start=True, stop=True)
                    rA = ost.tile([128, 1], F32, tag="r")
                    rB = ost.tile([128, 1], F32, tag="r")
                    nc.vector.reciprocal(rA, dA[:, 0:1])
                    nc.vector.reciprocal(rB, dB[:, 0:1])
                    t1 = ost.tile([128, NF], F32, tag="t")
                    nc.vector.tensor_scalar_mul(t1, oB, rB)
                    osb = ost.tile([128, NF], F32, tag="os")
                    nc.vector.scalar_tensor_tensor(osb, in0=oA, scalar=rA,
                                                   in1=t1, op0=ALU.mult,
                                                   op1=ALU.add)
                    nc.gpsimd.dma_start(o_d[sl, :], osb)

    nc.compile()
    return nc


def _prep_core_inputs(c, q, k, v, wq, bq, wk, bk, wv, patterns):
    b = c // 4
    hp = c % 4
    cols = slice(128 * hp, 128 * (hp + 1))
    e65 = np.zeros((65, 2), np.float32)
    e65[64, :] = 1.0
    n_pat = patterns.shape[0]
    wo_slice = _prep_core_inputs._wo[cols, :]  # [128, 512]
    return {
        "qT": np.ascontiguousarray(q[b].T),
        "kT": np.ascontiguousarray(k[b].T),
        "vT": np.ascontiguousarray(v[b].T),
        "wq": np.ascontiguousarray(wq[:, cols]),
        "wk": np.ascontiguousarray(wk[:, cols]),
        "wv": np.ascontiguousarray(wv[:, cols]),
        "wo": np.ascontiguousarray(
            wo_slice.reshape(2, 64, NF).transpose(1, 0, 2)),
        "bq": np.ascontiguousarray(bq[cols].reshape(128, 1)),
        "bk": np.ascontiguousarray(bk[cols].reshape(128, 1)),
        "e65": e65,
        "msk": np.ascontiguousarray(
            patterns.transpose(1, 0, 2).reshape(SK, n_pat * SQ)),
    }


def get_state(mask_np, reps=1):
    """Build (or fetch cached) compiled program + schedule for this mask."""
    mask2d = np.asarray(mask_np, dtype=np.float32).reshape(S, S)
    schedule, patterns = _classify_mask(mask2d)
    key = (schedule, patterns.tobytes(), reps)
    if key not in _CACHE:
        nc = _build_program(schedule, patterns.shape[0], reps=reps)
        _CACHE[key] = {"nc": nc, "schedule": schedule, "patterns": patterns}
    return _CACHE[key]


def kernel(q, k, v, mask, wq, bq, wk, bk, wv, bv, wo, bo):
    q = np.asarray(q, np.float32)
    k = np.asarray(k, np.float32)
    v = np.asarray(v, np.float32)
    wq_n = np.asarray(wq, np.float32)
    wk_n = np.asarray(wk, np.float32)
    wv_n = np.asarray(wv, np.float32)
    wo_n = np.asarray(wo, np.float32)
    bq_n = np.asarray(bq, np.float32)
    bk_n = np.asarray(bk, np.float32)
    bv_n = np.asarray(bv, np.float32)
    bo_n = np.asarray(bo, np.float32)

    state = get_state(mask)
    nc = state["nc"]
    patterns = state["patterns"]

    _prep_core_inputs._wo = wo_n
    in_maps = [
        _prep_core_inputs(c, q, k, v, wq_n, bq_n, wk_n, bk_n, wv_n, patterns)
        for c in range(N_CORES)
    ]
    results = bass2jax.run_bass_via_pjrt(nc, in_maps, n_cores=N_CORES)

    bo_eff = bv_n @ wo_n + bo_n  # exact: softmax rows sum to 1
    out = np.empty((B, S, NF), np.float32)
    for b in range(B):
        acc = results[b * 4 + 0]["o"].astype(np.float32)
        for hp in range(1, 4):
            acc = acc + results[b * 4 + hp]["o"]
        out[b] = acc + bo_eff
    return out


# revision 18
# speedup vs baseline: 1.2063x; 1.0171x over previous
"""Trainium2 Bass kernel for nn_MultiHeadAttention (B=2, S=4096, F=512, H=8, causal mask).

Sharding: 8 cores = 2 (batch) x 4 (head pairs). Each core computes the
projections for its 2 heads, causal flash-style attention with logits in
[Sk, Sq] (transposed) layout, and its partial output projection. The host
pre-transposes q/k/v per batch, slices weights per head pair, and sums the
4 partial outputs per batch afterwards (replaces the all-reduce). Biases are
exact: bq/bk applied on device during PSUM evacuation; bv/bo folded on host
as bv @ wo + bo (valid because softmax rows sum to 1).

The causal structure is not hardcoded blindly: the mask input is classified
on the host into full / partial / skipped [128 x 512] tiles and the device
program is built (and cached) from that schedule, so any 0/1-style additive
mask (including all-zeros) produces a correct program.

Numerics: matmuls run in float32r (tf32-like, full PE rate at free dim
>= 256); walrus requires f32r operands to be produced by rounding
instructions, hence the DVE/GPSIMD convert copies. Softmax uses
exp(logits/8 - 4) with no max pass (logits are O(6) for this problem size;
the -4 offset cancels exactly in the normalization). Denominators ride as a
ones-column in the PV stationary operand and are extracted per S-tile with a
basis-vector matmul (N=2 because f32r requires even free counts).
"""

import numpy as np
from contextlib import ExitStack

import concourse.bass as bass
import concourse.tile as tile
from concourse import bacc, mybir
from concourse import bass2jax

F32 = mybir.dt.float32
F32R = mybir.dt.float32r
BF16 = mybir.dt.bfloat16
AF = mybir.ActivationFunctionType
ALU = mybir.AluOpType

B = 2
S = 4096
NF = 512
NH = 8
D = 64
N_CORES = 8
SQ = 512          # query block width
SK = 128          # key tile height
N_QB = S // SQ    # 8
N_SKT = S // SK   # 32
N_ST = S // 128   # 32 S-tiles for projections / output
SCALE = 1.0 / np.sqrt(np.float32(D))  # 0.125
EXP_BIAS = -4.0   # constant shift inside exp; cancels exactly in normalization

_CACHE: dict = {}


def _classify_mask(mask: np.ndarray):
    """mask: [S, S] additive-style (nonzero => disallowed).

    Returns (schedule, patterns):
      schedule[qb] = list of (sk, qlo, pat_idx_or_None)
      patterns: np.ndarray [n_pat, 128, 512] of multiplicative 0/1 masks.
    """
    m = mask != 0  # True => masked out; indexed [q, k] per the reference
    schedule = []
    patterns = []
    pat_index: dict = {}
    for qb in range(N_QB):
        items = []
        for sk in range(N_SKT):
            # tile in [k, q] orientation to match the on-chip [Sk, Sq] layout
            sub = m[qb * SQ:(qb + 1) * SQ, sk * SK:(sk + 1) * SK].T
            if sub.all():
                continue
            if not sub.any():
                items.append((sk, 0, None))
                continue
            col_full_masked = sub.all(axis=0)
            # first column that is not fully masked
            qlo = int(np.argmax(~col_full_masked))
            # round down to multiple of 128 to keep matmul free dims >= 256-ish
            qlo = (qlo // 128) * 128
            pat = (~sub).astype(np.float32)  # 1 = allowed
            key = pat.tobytes()
            if key not in pat_index:
                pat_index[key] = len(patterns)
                patterns.append(pat)
            items.append((sk, qlo, pat_index[key]))
        schedule.append(tuple(items))
    pats = np.stack(patterns) if patterns else np.ones((1, SK, SQ), np.float32)
    return tuple(schedule), pats


def _group_units(items):
    """Pair up consecutive full tiles for 2-bank exp ops; partial tiles single."""
    units = []
    i = 0
    while i < len(items):
        if (i + 1 < len(items) and items[i][1] == 0 and items[i][2] is None
                and items[i + 1][1] == 0 and items[i + 1][2] is None):
            units.append((items[i], items[i + 1]))
            i += 2
        else:
            units.append((items[i],))
            i += 1
    return units


def _build_program(schedule, n_pat, reps=1):
    nc = bacc.Bacc("TRN2", target_bir_lowering=False, debug=False,
                   num_devices=N_CORES)

    qT = nc.dram_tensor("qT", [NF, S], F32, kind="ExternalInput").ap()
    kT = nc.dram_tensor("kT", [NF, S], F32, kind="ExternalInput").ap()
    vT = nc.dram_tensor("vT", [NF, S], F32, kind="ExternalInput").ap()
    wq_d = nc.dram_tensor("wq", [NF, 128], F32, kind="ExternalInput").ap()
    wk_d = nc.dram_tensor("wk", [NF, 128], F32, kind="ExternalInput").ap()
    wv_d = nc.dram_tensor("wv", [NF, 128], F32, kind="ExternalInput").ap()
    wo_d = nc.dram_tensor("wo", [64, 2, NF], F32, kind="ExternalInput").ap()
    bq_d = nc.dram_tensor("bq", [128, 1], F32, kind="ExternalInput").ap()
    bk_d = nc.dram_tensor("bk", [128, 1], F32, kind="ExternalInput").ap()
    e65_d = nc.dram_tensor("e65", [65, 2], F32, kind="ExternalInput").ap()
    msk_d = nc.dram_tensor("msk", [SK, n_pat * SQ], F32, kind="ExternalInput").ap()
    o_d = nc.dram_tensor("o", [S, NF], F32, kind="ExternalOutput").ap()

    with tile.TileContext(nc) as tc, ExitStack() as octx:
        per = octx.enter_context(tc.tile_pool(name="persist", bufs=1))

        QhT = per.tile([128, S], F32R, tag="qh")      # [head dims (A|B), S]
        KhT = per.tile([128, S], F32R, tag="kh")
        Vaug = per.tile([128, N_SKT, 132], F32R, tag="vaug")  # [A(64)|1|B(64)|1|pad2]
        attnA = per.tile([65, S], F32R, tag="attnA")  # rows 0-63 attn, row 64 denom
        attnB = per.tile([65, S], F32R, tag="attnB")
        wq_st = per.tile([128, 4, 128], F32, tag="wq_st")
        wk_st = per.tile([128, 4, 128], F32, tag="wk_st")
        wq_sb = per.tile([128, 4, 128], F32R, tag="wq")
        wk_sb = per.tile([128, 4, 128], F32R, tag="wk")
        wv_sb = per.tile([128, 4, 128], F32, tag="wv")
        wo_st = per.tile([64, 2, NF], F32, tag="wo_st")
        wo_sb = per.tile([64, 2, NF], F32R, tag="wo")
        bq_sb = per.tile([128, 1], F32, tag="bq")
        bk_sb = per.tile([128, 1], F32, tag="bk")
        e65_st = per.tile([65, 2], F32, tag="e65_st")
        e65_sb = per.tile([65, 2], F32R, tag="e65")
        msk_sb = per.tile([SK, n_pat * SQ], F32, tag="msk")
        ebias = per.tile([128, 1], F32, tag="ebias")
        vones = per.tile([128, N_SKT, 132], F32, tag="vones")

        nc.vector.memset(ebias, EXP_BIAS)
        nc.vector.memset(vones, 1.0)
        nc.vector.tensor_copy(Vaug, vones)
        nc.sync.dma_start(wq_st, wq_d.rearrange("(c p) m -> p c m", p=128))
        nc.sync.dma_start(wk_st, wk_d.rearrange("(c p) m -> p c m", p=128))
        nc.sync.dma_start(wv_sb, wv_d.rearrange("(c p) m -> p c m", p=128))
        nc.vector.tensor_copy(wq_sb, wq_st)
        nc.vector.tensor_copy(wk_sb, wk_st)
        nc.sync.dma_start(bq_sb, bq_d)
        nc.sync.dma_start(bk_sb, bk_d)
        nc.sync.dma_start(msk_sb, msk_d)
        nc.sync.dma_start(wo_st, wo_d)
        nc.vector.tensor_copy(wo_sb, wo_st)
        nc.sync.dma_start(e65_st, e65_d)
        nc.vector.tensor_copy(e65_sb, e65_st)

        for _rep in range(reps):
            # Main pipeline: per query block, project K/Q/V then attention.
            # PSUM banks: qk 1 + v 1 + lt 2x[128,1024]=4 + pv 2 = 8.
            with tc.tile_pool(name="xs", bufs=2) as xs, \
                 tc.tile_pool(name="psqk", bufs=2, space="PSUM") as psqk, \
                 tc.tile_pool(name="pp", bufs=4) as pp, \
                 tc.tile_pool(name="ltp", bufs=2, space="PSUM") as ltp, \
                 tc.tile_pool(name="pvp", bufs=2, space="PSUM") as pvp:
                for qb in range(N_QB):
                    qsl = slice(qb * SQ, (qb + 1) * SQ)
                    q0 = qb * SQ
                    # K/Q projections: one merged 1MB load + one rounding
                    # copy per tensor per block
                    for dst, src, w_s, b_s in ((KhT, kT, wk_sb, bk_sb),
                                               (QhT, qT, wq_sb, bq_sb)):
                        pt = psqk.tile([128, SQ], F32, tag="qk")
                        xb = xs.tile([128, 4, SQ], F32, tag="x", bufs=3)
                        nc.sync.dma_start(
                            xb, src.rearrange("(c p) m -> p c m", p=128)[:, :, qsl])
                        xr = xs.tile([128, 4, SQ], F32R, tag="xr", bufs=3)
                        nc.vector.tensor_copy(xr, xb)
                        for f in range(4):
                            nc.tensor.matmul(pt, w_s[:, f, :], xr[:, f, :],
                                             start=(f == 0), stop=(f == 3))
                        nc.vector.tensor_scalar_add(dst[:, qsl], pt, b_s)
                    # V projection: one 512-wide load per f-chunk, then the
                    # four 128-wide stationaries slice it (plain f32)
                    vbig = xs.tile([128, 4, SQ], F32, tag="vx")
                    nc.gpsimd.dma_start(
                        vbig, vT.rearrange("(c p) m -> p c m", p=128)[:, :, qsl])
                    for j in range(4):
                        st = 4 * qb + j
                        pv_ = psqk.tile([128, 128], F32, tag="qk")
                        for f in range(4):
                            nc.tensor.matmul(pv_, vbig[:, f, j * 128:(j + 1) * 128],
                                             wv_sb[:, f, :],
                                             start=(f == 0), stop=(f == 3))
                        nc.vector.tensor_copy(
                            Vaug[:, st, 0:130].rearrange(
                                "p (two x) -> p two x", x=65)[:, :, 0:64],
                            pv_.rearrange("p (two x) -> p two x", x=64))
                    # Attention for this block
                    items = schedule[qb]
                    if not items:
                        continue
                    pvA = pvp.tile([65, SQ], F32, tag="pv")
                    pvB = pvp.tile([65, SQ], F32, tag="pv")
                    n_items = len(items)
                    for idx, (sk, qlo, pat) in enumerate(items):
                        ksl = slice(sk * SK, (sk + 1) * SK)
                        qs = slice(q0 + qlo, q0 + SQ)
                        lt = ltp.tile([128, 1024], F32, tag="lt")
                        pAB = pp.tile([128, 1024], F32R, tag="pAB")
                        nc.tensor.matmul(lt[:, qlo:SQ], KhT[0:64, ksl],
                                         QhT[0:64, qs], start=True, stop=True)
                        nc.tensor.matmul(lt[:, SQ + qlo:2 * SQ], KhT[64:128, ksl],
                                         QhT[64:128, qs], start=True, stop=True)
                        if qlo == 0:
                            nc.scalar.activation(pAB, lt, AF.Exp,
                                                 bias=ebias, scale=float(SCALE))
                        else:
                            oap = pAB.rearrange("p (two q) -> p two q",
                                                q=SQ)[:, :, qlo:SQ]
                            iap = lt.rearrange("p (two q) -> p two q",
                                               q=SQ)[:, :, qlo:SQ]
                            nc.scalar.activation(oap, iap, AF.Exp,
                                                 bias=ebias, scale=float(SCALE))
                        if pat is not None:
                            msl = msk_sb[:, pat * SQ + qlo:(pat + 1) * SQ].bitcast(F32R)
                            nc.vector.tensor_mul(pAB[:, qlo:SQ],
                                                 pAB[:, qlo:SQ], msl)
                            nc.vector.tensor_mul(pAB[:, SQ + qlo:2 * SQ],
                                                 pAB[:, SQ + qlo:2 * SQ], msl)
                        st_flag = (idx == 0)
                        sp_flag = (idx == n_items - 1)
                        nc.tensor.matmul(pvA[:, qlo:SQ], Vaug[:, sk, 0:65],
                                         pAB[:, qlo:SQ],
                                         start=st_flag, stop=sp_flag)
                        nc.tensor.matmul(pvB[:, qlo:SQ], Vaug[:, sk, 65:130],
                                         pAB[:, SQ + qlo:2 * SQ],
                                         start=st_flag, stop=sp_flag)
                    nc.vector.tensor_copy(attnA[:, qsl], pvA)
                    nc.vector.tensor_copy(attnB[:, qsl], pvB)

            # Output projection
            with tc.tile_pool(name="ost", bufs=4) as ost, \
                 tc.tile_pool(name="pso", bufs=4, space="PSUM") as pso, \
                 tc.tile_pool(name="psd", bufs=4, space="PSUM") as psd:
                for st in range(N_ST):
                    sl = slice(st * 128, (st + 1) * 128)
                    oA = pso.tile([128, NF], F32, tag="o")
                    oB = pso.tile([128, NF], F32, tag="o")
                    dA = psd.tile([128, 2], F32, tag="d")
                    dB = psd.tile([128, 2], F32, tag="d")
                    nc.tensor.matmul(oA, attnA[0:64, sl], wo_sb[:, 0, :],
                                     start=True, stop=True)
                    nc.tensor.matmul(oB, attnB[0:64, sl], wo_sb[:, 1, :],
                                     start=True, stop=True)
                    nc.tensor.matmul(dA, attnA[0:65, sl], e65_sb,
                                     start=True, stop=True)
                    nc.tensor.matmul(dB, attnB[0:65, sl], e65_sb,
                                     start=True, stop=True)
                    rA = ost.tile([128, 1], F32, tag="r")
                    rB = ost.tile([128, 1], F32, tag="r")
                    nc.vector.reciprocal(rA, dA[:, 0:1])
                    nc.vector.reciprocal(rB, dB[:, 0:1])
                    t1 = ost.tile([128, NF], F32, tag="t")
                    nc.vector.tensor_scalar_mul(t1, oB, rB)
                    osb = ost.tile([128, NF], F32, tag="os")
                    nc.vector.scalar_tensor_tensor(osb, in0=oA, scalar=rA,
                                                   in1=t1, op0=ALU.mult,
                                                   op1=ALU.add)
                    nc.gpsimd.dma_start(o_d[sl, :], osb)

    nc.compile()
    return nc


def _prep_core_inputs(c, q, k, v, wq, bq, wk, bk, wv, patterns):
    b = c // 4
    hp = c % 4
    cols = slice(128 * hp, 128 * (hp + 1))
    e65 = np.zeros((65, 2), np.float32)
    e65[64, :] = 1.0
    n_pat = patterns.shape[0]
    wo_slice = _prep_core_inputs._wo[cols, :]  # [128, 512]
    return {
        "qT": np.ascontiguousarray(q[b].T),
        "kT": np.ascontiguousarray(k[b].T),
        "vT": np.ascontiguousarray(v[b].T),
        "wq": np.ascontiguousarray(wq[:, cols]),
        "wk": np.ascontiguousarray(wk[:, cols]),
        "wv": np.ascontiguousarray(wv[:, cols]),
        "wo": np.ascontiguousarray(
            wo_slice.reshape(2, 64, NF).transpose(1, 0, 2)),
        "bq": np.ascontiguousarray(bq[cols].reshape(128, 1)),
        "bk": np.ascontiguousarray(bk[cols].reshape(128, 1)),
        "e65": e65,
        "msk": np.ascontiguousarray(
            patterns.transpose(1, 0, 2).reshape(SK, n_pat * SQ)),
    }


def get_state(mask_np, reps=1):
    """Build (or fetch cached) compiled program + schedule for this mask."""
    mask2d = np.asarray(mask_np, dtype=np.float32).reshape(S, S)
    schedule, patterns = _classify_mask(mask2d)
    key = (schedule, patterns.tobytes(), reps)
    if key not in _CACHE:
        nc = _build_program(schedule, patterns.shape[0], reps=reps)
        _CACHE[key] = {"nc": nc, "schedule": schedule, "patterns": patterns}
    return _CACHE[key]


def kernel(q, k, v, mask, wq, bq, wk, bk, wv, bv, wo, bo):
    q = np.asarray(q, np.float32)
    k = np.asarray(k, np.float32)
    v = np.asarray(v, np.float32)
    wq_n = np.asarray(wq, np.float32)
    wk_n = np.asarray(wk, np.float32)
    wv_n = np.asarray(wv, np.float32)
    wo_n = np.asarray(wo, np.float32)
    bq_n = np.asarray(bq, np.float32)
    bk_n = np.asarray(bk, np.float32)
    bv_n = np.asarray(bv, np.float32)
    bo_n = np.asarray(bo, np.float32)

    state = get_state(mask)
    nc = state["nc"]
    patterns = state["patterns"]

    _prep_core_inputs._wo = wo_n
    in_maps = [
        _prep_core_inputs(c, q, k, v, wq_n, bq_n, wk_n, bk_n, wv_n, patterns)
        for c in range(N_CORES)
    ]
    results = bass2jax.run_bass_via_pjrt(nc, in_maps, n_cores=N_CORES)

    bo_eff = bv_n @ wo_n + bo_n  # exact: softmax rows sum to 1
    out = np.empty((B, S, NF), np.float32)
    for b in range(B):
        acc = results[b * 4 + 0]["o"].astype(np.float32)
        for hp in range(1, 4):
            acc = acc + results[b * 4 + hp]["o"]
        out[b] = acc + bo_eff
    return out


# revision 19
# speedup vs baseline: 1.2462x; 1.0331x over previous
"""Trainium2 Bass kernel for nn_MultiHeadAttention (B=2, S=4096, F=512, H=8, causal mask).

Sharding: 8 cores = 2 (batch) x 4 (head pairs). Each core computes the
projections for its 2 heads, causal flash-style attention with logits in
[Sk, Sq] (transposed) layout, and its partial output projection. The host
pre-transposes q/k/v per batch, slices weights per head pair, and sums the
4 partial outputs per batch afterwards (replaces the all-reduce). Biases are
exact: bq/bk applied on device during PSUM evacuation; bv/bo folded on host
as bv @ wo + bo (valid because softmax rows sum to 1).

The causal structure is not hardcoded blindly: the mask input is classified
on the host into full / partial / skipped [128 x 512] tiles and the device
program is built (and cached) from that schedule, so any 0/1-style additive
mask (including all-zeros) produces a correct program.

Numerics: matmuls run in float32r (tf32-like, full PE rate at free dim
>= 256); walrus requires f32r operands to be produced by rounding
instructions, hence the DVE/GPSIMD convert copies. Softmax uses
exp(logits/8 - 4) with no max pass (logits are O(6) for this problem size;
the -4 offset cancels exactly in the normalization). Denominators ride as a
ones-column in the PV stationary operand and are extracted per S-tile with a
basis-vector matmul (N=2 because f32r requires even free counts).
"""

import numpy as np
from contextlib import ExitStack

import concourse.bass as bass
import concourse.tile as tile
from concourse import bacc, mybir
from concourse import bass2jax

F32 = mybir.dt.float32
F32R = mybir.dt.float32r
BF16 = mybir.dt.bfloat16
AF = mybir.ActivationFunctionType
ALU = mybir.AluOpType

B = 2
S = 4096
NF = 512
NH = 8
D = 64
N_CORES = 8
SQ = 512          # query block width
SK = 128          # key tile height
N_QB = S // SQ    # 8
N_SKT = S // SK   # 32
N_ST = S // 128   # 32 S-tiles for projections / output
SCALE = 1.0 / np.sqrt(np.float32(D))  # 0.125
EXP_BIAS = -4.0   # constant shift inside exp; cancels exactly in normalization

_CACHE: dict = {}


def _classify_mask(mask: np.ndarray):
    """mask: [S, S] additive-style (nonzero => disallowed).

    Returns (schedule, patterns):
      schedule[qb] = list of (sk, qlo, pat_idx_or_None)
      patterns: np.ndarray [n_pat, 128, 512] of multiplicative 0/1 masks.
    """
    m = mask != 0  # True => masked out; indexed [q, k] per the reference
    schedule = []
    patterns = []
    pat_index: dict = {}
    for qb in range(N_QB):
        items = []
        for sk in range(N_SKT):
            # tile in [k, q] orientation to match the on-chip [Sk, Sq] layout
            sub = m[qb * SQ:(qb + 1) * SQ, sk * SK:(sk + 1) * SK].T
            if sub.all():
                continue
            if not sub.any():
                items.append((sk, 0, None))
                continue
            col_full_masked = sub.all(axis=0)
            # first column that is not fully masked
            qlo = int(np.argmax(~col_full_masked))
            # round down to multiple of 128 to keep matmul free dims >= 256-ish
            qlo = (qlo // 128) * 128
            pat = (~sub).astype(np.float32)  # 1 = allowed
            key = pat.tobytes()
            if key not in pat_index:
                pat_index[key] = len(patterns)
                patterns.append(pat)
            items.append((sk, qlo, pat_index[key]))
        schedule.append(tuple(items))
    pats = np.stack(patterns) if patterns else np.ones((1, SK, SQ), np.float32)
    return tuple(schedule), pats


def _group_units(items):
    """Pair up consecutive full tiles for 2-bank exp ops; partial tiles single."""
    units = []
    i = 0
    while i < len(items):
        if (i + 1 < len(items) and items[i][1] == 0 and items[i][2] is None
                and items[i + 1][1] == 0 and items[i + 1][2] is None):
            units.append((items[i], items[i + 1]))
            i += 2
        else:
            units.append((items[i],))
            i += 1
    return units


def _build_program(schedule, n_pat, reps=1):
    nc = bacc.Bacc("TRN2", target_bir_lowering=False, debug=False,
                   num_devices=N_CORES)

    qT = nc.dram_tensor("qT", [NF, S], F32, kind="ExternalInput").ap()
    kT = nc.dram_tensor("kT", [NF, S], F32, kind="ExternalInput").ap()
    vT = nc.dram_tensor("vT", [NF, S], F32, kind="ExternalInput").ap()
    wq_d = nc.dram_tensor("wq", [NF, 128], F32, kind="ExternalInput").ap()
    wk_d = nc.dram_tensor("wk", [NF, 128], F32, kind="ExternalInput").ap()
    wv_d = nc.dram_tensor("wv", [NF, 128], F32, kind="ExternalInput").ap()
    wo_d = nc.dram_tensor("wo", [64, 2, NF], F32, kind="ExternalInput").ap()
    bq_d = nc.dram_tensor("bq", [128, 1], F32, kind="ExternalInput").ap()
    bk_d = nc.dram_tensor("bk", [128, 1], F32, kind="ExternalInput").ap()
    e65_d = nc.dram_tensor("e65", [65, 2], F32, kind="ExternalInput").ap()
    msk_d = nc.dram_tensor("msk", [SK, n_pat * SQ], F32, kind="ExternalInput").ap()
    o_d = nc.dram_tensor("o", [S, NF], F32, kind="ExternalOutput").ap()

    with tile.TileContext(nc) as tc, ExitStack() as octx:
        per = octx.enter_context(tc.tile_pool(name="persist", bufs=1))

        QhT = per.tile([128, S], F32R, tag="qh")      # [head dims (A|B), S]
        KhT = per.tile([128, S], F32R, tag="kh")
        Vaug = per.tile([128, N_SKT, 132], F32R, tag="vaug")  # [A(64)|1|B(64)|1|pad2]
        attnA = per.tile([65, S], F32R, tag="attnA")  # rows 0-63 attn, row 64 denom
        attnB = per.tile([65, S], F32R, tag="attnB")
        wq_st = per.tile([128, 4, 128], F32, tag="wq_st")
        wk_st = per.tile([128, 4, 128], F32, tag="wk_st")
        wq_sb = per.tile([128, 4, 128], F32R, tag="wq")
        wk_sb = per.tile([128, 4, 128], F32R, tag="wk")
        wv_sb = per.tile([128, 4, 128], F32, tag="wv")
        wo_st = per.tile([64, 2, NF], F32, tag="wo_st")
        wo_sb = per.tile([64, 2, NF], F32R, tag="wo")
        bq_sb = per.tile([128, 1], F32, tag="bq")
        bk_sb = per.tile([128, 1], F32, tag="bk")
        e65_st = per.tile([65, 2], F32, tag="e65_st")
        e65_sb = per.tile([65, 2], F32R, tag="e65")
        msk_sb = per.tile([SK, n_pat * SQ], F32, tag="msk")
        ebias = per.tile([128, 1], F32, tag="ebias")
        vones = per.tile([128, N_SKT, 132], F32, tag="vones")

        nc.vector.memset(ebias, EXP_BIAS)
        nc.vector.memset(vones, 1.0)
        nc.vector.tensor_copy(Vaug, vones)
        nc.sync.dma_start(wq_st, wq_d.rearrange("(c p) m -> p c m", p=128))
        nc.sync.dma_start(wk_st, wk_d.rearrange("(c p) m -> p c m", p=128))
        nc.sync.dma_start(wv_sb, wv_d.rearrange("(c p) m -> p c m", p=128))
        nc.vector.tensor_copy(wq_sb, wq_st)
        nc.vector.tensor_copy(wk_sb, wk_st)
        nc.sync.dma_start(bq_sb, bq_d)
        nc.sync.dma_start(bk_sb, bk_d)
        nc.sync.dma_start(msk_sb, msk_d)
        nc.sync.dma_start(wo_st, wo_d)
        nc.vector.tensor_copy(wo_sb, wo_st)
        nc.sync.dma_start(e65_st, e65_d)
        nc.vector.tensor_copy(e65_sb, e65_st)

        for _rep in range(reps):
            # Main pipeline: per query block, project K/Q/V then attention.
            # PSUM banks: qk 1 + v 1 + lt 2x[128,1024]=4 + pv 2 = 8.
            with tc.tile_pool(name="xs", bufs=2) as xs, \
                 tc.tile_pool(name="psqk", bufs=2, space="PSUM") as psqk, \
                 tc.tile_pool(name="pp", bufs=4) as pp, \
                 tc.tile_pool(name="ltp", bufs=2, space="PSUM") as ltp, \
                 tc.tile_pool(name="pvp", bufs=2, space="PSUM") as pvp:
                def emit_proj(qb):
                    qsl = slice(qb * SQ, (qb + 1) * SQ)
                    # K/Q projections: one merged 1MB load + one rounding
                    # copy per tensor per block
                    for dst, src, w_s, b_s in ((KhT, kT, wk_sb, bk_sb),
                                               (QhT, qT, wq_sb, bq_sb)):
                        pt = psqk.tile([128, SQ], F32, tag="qk")
                        xb = xs.tile([128, 4, SQ], F32, tag="x", bufs=3)
                        nc.sync.dma_start(
                            xb, src.rearrange("(c p) m -> p c m", p=128)[:, :, qsl])
                        xr = xs.tile([128, 4, SQ], F32R, tag="xr", bufs=3)
                        nc.vector.tensor_copy(xr, xb)
                        for f in range(4):
                            nc.tensor.matmul(pt, w_s[:, f, :], xr[:, f, :],
                                             start=(f == 0), stop=(f == 3))
                        nc.vector.tensor_scalar_add(dst[:, qsl], pt, b_s)
                    # V projection: one 2MB load, four 128-wide stationaries
                    vbig = xs.tile([128, 4, SQ], F32, tag="vx")
                    nc.gpsimd.dma_start(
                        vbig, vT.rearrange("(c p) m -> p c m", p=128)[:, :, qsl])
                    for j in range(4):
                        st = 4 * qb + j
                        pv_ = psqk.tile([128, 128], F32, tag="qk")
                        for f in range(4):
                            nc.tensor.matmul(pv_, vbig[:, f, j * 128:(j + 1) * 128],
                                             wv_sb[:, f, :],
                                             start=(f == 0), stop=(f == 3))
                        nc.vector.tensor_copy(
                            Vaug[:, st, 0:130].rearrange(
                                "p (two x) -> p two x", x=65)[:, :, 0:64],
                            pv_.rearrange("p (two x) -> p two x", x=64))

                def emit_attn(qb):
                    qsl = slice(qb * SQ, (qb + 1) * SQ)
                    q0 = qb * SQ
                    items = schedule[qb]
                    if not items:
                        return
                    pvA = pvp.tile([65, SQ], F32, tag="pv")
                    pvB = pvp.tile([65, SQ], F32, tag="pv")
                    n_items = len(items)
                    for idx, (sk, qlo, pat) in enumerate(items):
                        ksl = slice(sk * SK, (sk + 1) * SK)
                        qs = slice(q0 + qlo, q0 + SQ)
                        lt = ltp.tile([128, 1024], F32, tag="lt")
                        pAB = pp.tile([128, 1024], F32R, tag="pAB")
                        nc.tensor.matmul(lt[:, qlo:SQ], KhT[0:64, ksl],
                                         QhT[0:64, qs], start=True, stop=True)
                        nc.tensor.matmul(lt[:, SQ + qlo:2 * SQ], KhT[64:128, ksl],
                                         QhT[64:128, qs], start=True, stop=True)
                        if qlo == 0:
                            nc.scalar.activation(pAB, lt, AF.Exp,
                                                 bias=ebias, scale=float(SCALE))
                        else:
                            oap = pAB.rearrange("p (two q) -> p two q",
                                                q=SQ)[:, :, qlo:SQ]
                            iap = lt.rearrange("p (two q) -> p two q",
                                               q=SQ)[:, :, qlo:SQ]
                            nc.scalar.activation(oap, iap, AF.Exp,
                                                 bias=ebias, scale=float(SCALE))
                        if pat is not None:
                            msl = msk_sb[:, pat * SQ + qlo:(pat + 1) * SQ].bitcast(F32R)
                            nc.vector.tensor_mul(pAB[:, qlo:SQ],
                                                 pAB[:, qlo:SQ], msl)
                            nc.vector.tensor_mul(pAB[:, SQ + qlo:2 * SQ],
                                                 pAB[:, SQ + qlo:2 * SQ], msl)
                        st_flag = (idx == 0)
                        sp_flag = (idx == n_items - 1)
                        nc.tensor.matmul(pvA[:, qlo:SQ], Vaug[:, sk, 0:65],
                                         pAB[:, qlo:SQ],
                                         start=st_flag, stop=sp_flag)
                        nc.tensor.matmul(pvB[:, qlo:SQ], Vaug[:, sk, 65:130],
                                         pAB[:, SQ + qlo:2 * SQ],
                                         start=st_flag, stop=sp_flag)
                    nc.vector.tensor_copy(attnA[:, qsl], pvA)
                    nc.vector.tensor_copy(attnB[:, qsl], pvB)

                # Software-pipelined emission: block qb+1's projections (and
                # their DMAs) are emitted before block qb's attention so the
                # scheduler prioritizes the prefetch.
                emit_proj(0)
                for qb in range(N_QB):
                    if qb + 1 < N_QB:
                        emit_proj(qb + 1)
                    emit_attn(qb)

            # Output projection
            with tc.tile_pool(name="ost", bufs=4) as ost, \
                 tc.tile_pool(name="pso", bufs=4, space="PSUM") as pso, \
                 tc.tile_pool(name="psd", bufs=4, space="PSUM") as psd:
                for st in range(N_ST):
                    sl = slice(st * 128, (st + 1) * 128)
                    oA = pso.tile([128, NF], F32, tag="o")
                    oB = pso.tile([128, NF], F32, tag="o")
                    dA = psd.tile([128, 2], F32, tag="d")
                    dB = psd.tile([128, 2], F32, tag="d")
                    nc.tensor.matmul(oA, attnA[0:64, sl], wo_sb[:, 0, :],
                                     start=True, stop=True)
                    nc.tensor.matmul(oB, attnB[0:64, sl], wo_sb[:, 1, :],
                                     start=True, stop=True)
                    nc.tensor.matmul(dA, attnA[0:65, sl], e65_sb,
                                     start=True, stop=True)
                    nc.tensor.matmul(dB, attnB[0:65, sl], e65_sb,
                                     start=True, stop=True)
                    rA = ost.tile([128, 1], F32, tag="r")
                    rB = ost.tile([128, 1], F32, tag="r")
                    nc.vector.reciprocal(rA, dA[:, 0:1])
                    nc.vector.reciprocal(rB, dB[:, 0:1])
                    t1 = ost.tile([128, NF], F32, tag="t")
                    nc.vector.tensor_scalar_mul(t1, oB, rB)
                    osb = ost.tile([128, NF], F32, tag="os")
                    nc.vector.scalar_tensor_tensor(osb, in0=oA, scalar=rA,
                                                   in1=t1, op0=ALU.mult,
                                                   op1=ALU.add)
                    nc.gpsimd.dma_start(o_d[sl, :], osb)

    nc.compile()
    return nc


def _prep_core_inputs(c, q, k, v, wq, bq, wk, bk, wv, patterns):
    b = c // 4
    hp = c % 4
    cols = slice(128 * hp, 128 * (hp + 1))
    e65 = np.zeros((65, 2), np.float32)
    e65[64, :] = 1.0
    n_pat = patterns.shape[0]
    wo_slice = _prep_core_inputs._wo[cols, :]  # [128, 512]
    return {
        "qT": np.ascontiguousarray(q[b].T),
        "kT": np.ascontiguousarray(k[b].T),
        "vT": np.ascontiguousarray(v[b].T),
        "wq": np.ascontiguousarray(wq[:, cols]),
        "wk": np.ascontiguousarray(wk[:, cols]),
        "wv": np.ascontiguousarray(wv[:, cols]),
        "wo": np.ascontiguousarray(
            wo_slice.reshape(2, 64, NF).transpose(1, 0, 2)),
        "bq": np.ascontiguousarray(bq[cols].reshape(128, 1)),
        "bk": np.ascontiguousarray(bk[cols].reshape(128, 1)),
        "e65": e65,
        "msk": np.ascontiguousarray(
            patterns.transpose(1, 0, 2).reshape(SK, n_pat * SQ)),
    }


def get_state(mask_np, reps=1):
    """Build (or fetch cached) compiled program + schedule for this mask."""
    mask2d = np.asarray(mask_np, dtype=np.float32).reshape(S, S)
    schedule, patterns = _classify_mask(mask2d)
    key = (schedule, patterns.tobytes(), reps)
    if key not in _CACHE:
        nc = _build_program(schedule, patterns.shape[0], reps=reps)
        _CACHE[key] = {"nc": nc, "schedule": schedule, "patterns": patterns}
    return _CACHE[key]


def kernel(q, k, v, mask, wq, bq, wk, bk, wv, bv, wo, bo):
    q = np.asarray(q, np.float32)
    k = np.asarray(k, np.float32)
    v = np.asarray(v, np.float32)
    wq_n = np.asarray(wq, np.float32)
    wk_n = np.asarray(wk, np.float32)
    wv_n = np.asarray(wv, np.float32)
    wo_n = np.asarray(wo, np.float32)
    bq_n = np.asarray(bq, np.float32)
    bk_n = np.asarray(bk, np.float32)
    bv_n = np.asarray(bv, np.float32)
    bo_n = np.asarray(bo, np.float32)

    state = get_state(mask)
    nc = state["nc"]
    patterns = state["patterns"]

    _prep_core_inputs._wo = wo_n
    in_maps = [
        _prep_core_inputs(c, q, k, v, wq_n, bq_n, wk_n, bk_n, wv_n, patterns)
        for c in range(N_CORES)
    ]
    results = bass2jax.run_bass_via_pjrt(nc, in_maps, n_cores=N_CORES)

    bo_eff = bv_n @ wo_n + bo_n  # exact: softmax rows sum to 1
    out = np.empty((B, S, NF), np.float32)
    for b in range(B):
        acc = results[b * 4 + 0]["o"].astype(np.float32)
        for hp in range(1, 4):
            acc = acc + results[b * 4 + hp]["o"]
        out[b] = acc + bo_eff
    return out
